# revision 3
# baseline (speedup 1.0000x reference)
"""Trainium2 Bass kernel for nn_DSSConf — v2: zero collectives.

Design: replicate the full x to every core, ROTATED by the core's node
offset so the SPMD program is fully static (each core's own shard is
always at local offset 0). Each core computes the full xf table and the
full (replicated) GIN branch locally, then processes its 1/8 of the
conformer edges and emits its output shard. No collectives at all.

Self-contained: hardcodes shapes/sharding; exposes kernel(**inputs).
"""
import sys
import math
from dataclasses import dataclass

sys.path.insert(0, "/opt/trn_rl_repo")

import numpy as np
from concourse import bass, bacc, tile, mybir, bass_utils

F32 = mybir.dt.float32
BF16 = mybir.dt.bfloat16
I16 = mybir.dt.int16
ALU = mybir.AluOpType
ACTF = mybir.ActivationFunctionType
AX = mybir.AxisListType

WMLP_BF16 = False  # edge-filter MLP matmuls in bf16 (flip after precision exp)
GIN_BF16 = False


@dataclass(frozen=True)
class Dims:
    N: int = 100000        # conformer nodes
    H: int = 256           # hidden
    NF: int = 128          # num filters
    NG: int = 50           # num gaussians
    G: int = 10000         # graph nodes
    E: int = 1000000       # conformer edges
    EG: int = 30000        # graph edges
    VOCAB: int = 5
    CUTOFF: float = 10.0
    cores: int = 8
    qsize: int = 25000     # src quadrant size for int16 gather indices
    chunk: int = 2048      # conformer edge chunk (multiple of 128, divides EQ_pad)
    dcall: int = 1024      # max descriptors per gather/scatter DMA call
                           # (SWDGE ring = dynamic_dma_scratch_size/16 = 1024)
    gwin: int = 128        # GIN scatter window (<=128 segments)
    PW: int = 640          # padded GIN edges per window (multiple of 128)
    GW: int = 1            # GIN windows per gather call
    nchunk: int = 500      # node chunk for the h/out stage (divides NS, mult of 10)
    pch: int = 1280        # Phase A node chunk (mult of 128 and of rep=10)

    @property
    def NS(self):
        return self.N // self.cores

    @property
    def GS(self):
        return self.G // self.cores

    @property
    def NQ(self):
        return (self.N + self.qsize - 1) // self.qsize

    @property
    def phase_sizes(self):
        """Fixed per-quadrant edge-bucket sizes, one per dst-occurrence
        rank. Within a bucket every dst is unique -> dma_scatter_add calls
        that stay inside a bucket are race-free. Sized from a Poisson model
        with margin; host_prep asserts the actual counts fit."""
        lam = (self.E / (self.cores * self.NQ)) / self.NS
        sizes = []
        pmf = math.exp(-lam)
        cdf = pmf
        p = 0
        while True:
            sf = 1.0 - cdf  # P(X >= p+1)
            mp = self.NS * sf
            if mp < 0.02 and p > 0:
                break
            pad = int(mp + 4.0 * math.sqrt(max(mp, 1.0)) + 64)
            sizes.append((pad + 127) // 128 * 128)
            p += 1
            pmf *= lam / p
            cdf += pmf
            if p > 64:
                break
        tot = sum(sizes)
        eq = (tot + self.chunk - 1) // self.chunk * self.chunk
        sizes[-1] += eq - tot
        return sizes

    @property
    def EQ_pad(self):
        return sum(self.phase_sizes)

    @property
    def E_pad(self):
        return self.NQ * self.EQ_pad

    @property
    def NWIN(self):
        return (self.G + self.gwin - 1) // self.gwin

    @property
    def EG_pad(self):
        return self.NWIN * self.PW


REAL = Dims()


def _wrap16(arr, dtype=np.int16):
    """Edge i -> [i % 16, i // 16], replicated to 128 partitions."""
    a = np.asarray(arr).reshape(-1, 16).T.astype(dtype)
    return np.tile(a, (8, 1)).copy()


def _tile128(arr, dtype=np.float32):
    """Edge i -> [i % 128, i // 128] (per-partition scalar layout)."""
    return np.ascontiguousarray(np.asarray(arr).reshape(-1, 128).T.astype(dtype))


def host_prep(inputs, D: Dims):
    """Build per-core in_maps (list of dicts) for the SPMD kernel."""
    x = np.asarray(inputs["x"], np.float32)
    cnb = np.asarray(inputs["conf_node_batch"]).astype(np.int64)
    ei = np.asarray(inputs["edge_index_conf"]).astype(np.int64)
    ew = np.asarray(inputs["edge_weight_conf"], np.float32)
    ea = np.asarray(inputs["edge_attr_conf"], np.float32)
    eig = np.asarray(inputs["edge_index_graph"]).astype(np.int64)
    eag = np.asarray(inputs["edge_attr_graph"]).astype(np.int64)

    rep = D.N // D.G
    assert np.array_equal(cnb, np.repeat(np.arange(D.G), rep)), \
        "conf_node_batch structure mismatch"

    NS, GS = D.NS, D.GS
    src, dst = ei[0], ei[1]
    owner = dst // NS
    sizes = D.phase_sizes
    ph_off = np.concatenate([[0], np.cumsum(sizes)])[:-1]

    xT = np.ascontiguousarray(x.T).reshape(2, 128, D.N)

    # GIN edges (global)
    sg, dg = eig[0], eig[1]

    # ---- weights (replicated) ----
    H2 = D.H // 128
    w = {k: np.asarray(inputs[k], np.float32) for k in (
        "mlp_w1", "mlp_b1", "mlp_w2", "mlp_b2", "cf_lin1", "cf_lin2",
        "cf_lin2_b", "lin_w", "lin_b", "bond_emb", "gin_eps", "gin_w1",
        "gin_w2", "bn1_g", "bn1_b", "bn2_g", "bn2_b")}
    mmdt = np.float32
    const = dict(
        w1=w["mlp_w1"].astype(mmdt),                      # [NG, NF]
        b1col=w["mlp_b1"].reshape(D.NF, 1),
        w2=w["mlp_w2"].astype(mmdt),                      # [NF, NF]
        b2full=np.tile(w["mlp_b2"].reshape(1, D.NF), (128, 4)).copy(),
        lin1=np.ascontiguousarray(w["cf_lin1"].reshape(H2, 128, D.NF)),
        lin2=w["cf_lin2"],                                # [NF, H]
        lin2b=w["cf_lin2_b"].reshape(H2, 128, 1),
        linw=np.ascontiguousarray(w["lin_w"].reshape(H2, 128, D.H)),
        linb=w["lin_b"].reshape(H2, 128, 1),
        gw1=np.ascontiguousarray(w["gin_w1"].reshape(H2, 128, D.H)),
        gw2=np.ascontiguousarray(w["gin_w2"].reshape(H2, 128, D.H)),
        bondcat=np.ascontiguousarray(
            w["bond_emb"].reshape(3 * D.VOCAB, D.H)),
        bn1g=w["bn1_g"].reshape(H2, 128, 1), bn1b=w["bn1_b"].reshape(H2, 128, 1),
        bn2g=w["bn2_g"].reshape(H2, 128, 1), bn2b=w["bn2_b"].reshape(H2, 128, 1),
        epsv=np.full((128, 1), 1.0 + float(w["gin_eps"]), np.float32),
        zerocol=np.zeros((128, 1), np.float32),
        eps5col=np.full((128, 1), 1e-5, np.float32),
        pihalf=np.full((128, 1), -math.pi / 2, np.float32),
        iota=np.tile(np.arange(128, dtype=np.float32), (128, 1)).copy(),
        ident=np.eye(128, dtype=np.float32),
    )

    in_maps = []
    for c in range(D.cores):
        # ---- rotated full x ----
        off = c * NS
        xr = np.concatenate([xT[:, :, off:], xT[:, :, :off]], axis=2)

        # ---- conformer edges owned by this core ----
        sel = owner == c
        s = src[sel]
        dd = dst[sel] - off                      # [0, NS)
        sr = (s - off) % D.N                     # rotated src
        q = sr // D.qsize
        order = np.lexsort((dd, q))
        s_sr, s_d, s_q = sr[order], dd[order], q[order]
        s_w = ew[sel][order]
        s_a = ea[sel][order]

        src_pad = np.zeros(D.E_pad, np.int64)
        dst_pad = NS + (np.arange(D.E_pad, dtype=np.int64) % 128)
        w_pad = np.full(D.E_pad, D.CUTOFF, np.float32)   # C(CUTOFF)=0 kills pads
        a_pad = np.zeros((D.E_pad, D.NG), np.float32)
        bounds = np.searchsorted(s_q, np.arange(D.NQ + 1))
        for qq in range(D.NQ):
            lo, hi = bounds[qq], bounds[qq + 1]
            d = s_d[lo:hi]                        # dst-sorted within the bucket
            rank = np.arange(len(d)) - np.searchsorted(d, d)
            counts = np.bincount(rank, minlength=len(sizes))
            assert len(counts) <= len(sizes) and (counts <= sizes).all(), \
                f"phase overflow: core {c} quad {qq}: {counts} vs {sizes}"
            o = qq * D.EQ_pad
            pos = np.empty(len(d), np.int64)
            for p in np.unique(rank):
                psel = rank == p
                pos[psel] = o + ph_off[p] + np.arange(counts[p])
            src_pad[pos] = s_sr[lo:hi] - qq * D.qsize
            dst_pad[pos] = d
            w_pad[pos] = s_w[lo:hi]
            a_pad[pos] = s_a[lo:hi]

        # ---- GIN edges (full graph, rotated) ----
        sgr = (sg - c * GS) % D.G
        dgr = (dg - c * GS) % D.G
        gw_ = dgr // D.gwin
        gorder = np.lexsort((dgr, gw_))
        g_s, g_d, g_w = sgr[gorder], dgr[gorder], gw_[gorder]
        g_a = eag[gorder]

        sg_pad = np.zeros(D.EG_pad, np.int64)
        dr_pad = np.full(D.EG_pad, -1.0, np.float32)  # -1 kills pads in one-hot
        bh_pad = np.zeros((3 * D.VOCAB, D.EG_pad), np.float32)
        gbounds = np.searchsorted(g_w, np.arange(D.NWIN + 1))
        for ww in range(D.NWIN):
            lo, hi = gbounds[ww], gbounds[ww + 1]
            cnt = hi - lo
            assert cnt <= D.PW, f"PW overflow: core {c} win {ww}: {cnt}"
            o = ww * D.PW
            sg_pad[o:o + cnt] = g_s[lo:hi]
            dr_pad[o:o + cnt] = (g_d[lo:hi] - ww * D.gwin).astype(np.float32)
            for k in range(3):
                bh_pad[k * D.VOCAB + g_a[lo:hi, k], np.arange(o, o + cnt)] = 1.0

        m = dict(
            xT=np.ascontiguousarray(xr),
            AT=np.ascontiguousarray(a_pad.T),
            WT=_tile128(w_pad),
            SRC=_wrap16(src_pad),
            DSTI=_wrap16(dst_pad),
            SG=_wrap16(sg_pad),
            DREL=_tile128(dr_pad),
            BHOT=bh_pad,
        )
        m.update(const)
        in_maps.append(m)
    return in_maps


def assemble(results, D: Dims):
    """Per-core outT [2,128,NS] -> full [N, H]."""
    parts = [r["outT"].reshape(D.H, D.NS) for r in results]
    outT = np.concatenate(parts, axis=1)  # [H, N]
    return np.ascontiguousarray(outT.T)


def _ts(i, n):
    return bass.ts(i, n)


def build_nc(D: Dims, phases: str = "abcd"):
    nc = bacc.Bacc("TRN2", target_bir_lowering=False, debug=False,
                   num_devices=D.cores)
    NS, GS, H, NF, NG, G = D.NS, D.GS, D.H, D.NF, D.NG, D.G
    H2 = H // 128
    rep = D.N // D.G
    MMDT = F32

    I = {}
    def di(name, shape, dt=F32):
        I[name] = nc.dram_tensor(name, list(shape), dt, kind="ExternalInput")
        return I[name]

    di("xT", [2, 128, D.N])
    di("AT", [NG, D.E_pad])
    di("WT", [128, D.E_pad // 128])
    di("SRC", [128, D.E_pad // 16], I16)
    di("DSTI", [128, D.E_pad // 16], I16)
    di("SG", [128, D.EG_pad // 16], I16)
    di("DREL", [128, D.EG_pad // 128])
    di("BHOT", [3 * D.VOCAB, D.EG_pad])
    di("w1", [NG, NF], MMDT); di("b1col", [NF, 1]); di("w2", [NF, NF], MMDT)
    di("b2full", [128, 4 * NF])
    di("lin1", [H2, 128, NF]); di("lin2", [NF, H]); di("lin2b", [H2, 128, 1])
    di("linw", [H2, 128, H]); di("linb", [H2, 128, 1])
    di("gw1", [H2, 128, H]); di("gw2", [H2, 128, H])
    di("bondcat", [3 * D.VOCAB, H])
    di("bn1g", [H2, 128, 1]); di("bn1b", [H2, 128, 1])
    di("bn2g", [H2, 128, 1]); di("bn2b", [H2, 128, 1])
    di("epsv", [128, 1]); di("iota", [128, 128]); di("ident", [128, 128])
    di("zerocol", [128, 1]); di("eps5col", [128, 1]); di("pihalf", [128, 1])

    outT = nc.dram_tensor("outT", [2, 128, NS], F32, kind="ExternalOutput")

    with tile.TileContext(nc) as tc:
        with (
            tc.tile_pool(name="const", bufs=1) as cp,
            tc.tile_pool(name="work", bufs=2) as wp,
            tc.tile_pool(name="small", bufs=2) as sp,
            tc.tile_pool(name="psum", bufs=2, space="PSUM") as pp,
            tc.tile_pool(name="dram", bufs=1, space="DRAM") as dp,
        ):
            # ---------- load constants ----------
            C = {}
            for nm, shp, dt in [("w1", [NG, NF], MMDT), ("b1col", [NF, 1], F32),
                                ("w2", [NF, NF], MMDT),
                                ("b2full", [128, 4 * NF], F32),
                                ("lin2", [NF, H], F32),
                                ("bondcat", [3 * D.VOCAB, H], F32),
                                ("epsv", [128, 1], F32), ("iota", [128, 128], F32),
                                ("ident", [128, 128], F32),
                                ("zerocol", [128, 1], F32),
                                ("eps5col", [128, 1], F32),
                                ("pihalf", [128, 1], F32)]:
                t = cp.tile(shp, dt, name=f"c_{nm}")
                nc.sync.dma_start(t[:], I[nm].ap())
                C[nm] = t
            nc.const_aps.aps[(F32, 0.0)] = C["zerocol"][:]
            for nm in ("lin1", "lin2b", "linw", "linb", "gw1", "gw2",
                       "bn1g", "bn1b", "bn2g", "bn2b"):
                C[nm] = []
                inner = I[nm].shape[2]
                for k in range(H2):
                    t = cp.tile([128, inner], F32, name=f"c_{nm}{k}")
                    nc.sync.dma_start(t[:], I[nm].ap()[k])
                    C[nm].append(t)

            # ---------- DRAM scratch (all core-local) ----------
            xf_full = dp.tile([D.N, NF], F32, name="xf_full")
            xaggT_d = dp.tile([2, 128, G], F32, name="xaggT_d")
            xagg_full = dp.tile([G, H], F32, name="xagg_full")
            tT_d = dp.tile([2, 128, G], F32, name="tT_d")
            u1T_d = dp.tile([2, 128, G], F32, name="u1T_d")
            u2T_d = dp.tile([2, 128, G], F32, name="u2T_d")
            agg_dram = dp.tile([NS + 128, NF], F32, name="agg_dram")

            # =========== Phase A: xf for ALL nodes + segment-max pool =======
            # Full chunks of PCH (mult of 128 and rep), ragged tail after.
            PCH = D.pch
            n_full = D.N // PCH if "a" in phases else 0
            NTA = PCH // 128
            for j in range(n_full):
                xt = [wp.tile([128, PCH], F32, tag=f"ph_a_xt{k}",
                              name=f"ph_a_xt{k}")
                      for k in range(2)]
                for k in range(2):
                    nc.sync.dma_start(xt[k][:], I["xT"].ap()[k, :, _ts(j, PCH)])
                # pool: max over groups of `rep` cols -> xaggT_d chunk cols
                for k in range(2):
                    xa_sb = sp.tile([128, PCH // rep], F32, tag="ph_a_poolsb",
                                    name="ph_a_poolsb")
                    nc.vector.tensor_reduce(
                        xa_sb[:],
                        xt[k][:].rearrange("p (g t) -> p g t", t=rep),
                        AX.X, ALU.max)
                    nc.sync.dma_start(
                        xaggT_d[k][:, _ts(j, PCH // rep)], xa_sb[:])
                # xf = x @ cf_lin1: mm pairs into 4-tile psum banks, one
                # bulk row-wrapped write per chunk
                sb = wp.tile([128, NTA, NF], F32, tag="ph_a_sb", name="ph_a_sb")
                for g0 in range(0, NTA, 4):
                    gn = min(4, NTA - g0)
                    ps = pp.tile([128, 4, NF], F32, tag="ps_mm", name="ps_mm")
                    for ti in range(gn):
                        t = g0 + ti
                        for k in range(2):
                            nc.tensor.matmul(ps[:, ti, :],
                                             xt[k][:, t * 128:(t + 1) * 128],
                                             C["lin1"][k][:], start=(k == 0),
                                             stop=(k == 1))
                    nc.scalar.copy(sb[:, g0:g0 + gn, :], ps[:, :gn, :])
                nc.scalar.dma_start(
                    xf_full[j * PCH:(j + 1) * PCH, :].rearrange(
                        "(t p) f -> p t f", p=128),
                    sb[:])
            # ragged tail (node-at-a-time tiles)
            tail0 = n_full * PCH
            n_tail = D.N - tail0 if "a" in phases else 0
            if n_tail:
                xt = [wp.tile([128, n_tail], F32, tag=f"ph_a_xt{k}",
                              name=f"ph_a_xtt{k}")
                      for k in range(2)]
                for k in range(2):
                    nc.sync.dma_start(xt[k][:],
                                      I["xT"].ap()[k, :, tail0:tail0 + n_tail])
                for k in range(2):
                    xa_sb = sp.tile([128, n_tail // rep], F32,
                                    tag="ph_a_poolsb", name="ph_a_poolsbt")
                    nc.vector.tensor_reduce(
                        xa_sb[:],
                        xt[k][:].rearrange("p (g t) -> p g t", t=rep),
                        AX.X, ALU.max)
                    nc.sync.dma_start(
                        xaggT_d[k][:, tail0 // rep:(tail0 + n_tail) // rep],
                        xa_sb[:])
                for t in range((n_tail + 127) // 128):
                    m = min(128, n_tail - t * 128)
                    ps = pp.tile([128, 4, NF], F32, tag="ps_mm", name="ps_mm")
                    for k in range(2):
                        nc.tensor.matmul(ps[:m, 0, :],
                                         xt[k][:, t * 128:t * 128 + m],
                                         C["lin1"][k][:], start=(k == 0),
                                         stop=(k == 1))
                    sb = sp.tile([128, NF], F32, tag="ph_a_tsb", name="ph_a_tsb")
                    nc.scalar.copy(sb[:m, :], ps[:m, 0, :])
                    nc.sync.dma_start(
                        xf_full[tail0 + t * 128: tail0 + t * 128 + m, :],
                        sb[:m, :])

            # =========== Phase A2: transpose xagg -> node-major =============
            GT = (G + 127) // 128
            for t in range(GT if "a" in phases else 0):
                m = min(128, G - t * 128)
                for k in range(2):
                    la = sp.tile([128, 128], F32, tag="ph_a2_la", name="ph_a2_la")
                    nc.sync.dma_start(la[:, :m],
                                      xaggT_d[k][:, t * 128:t * 128 + m])
                    pst = pp.tile([128, 128], F32, tag="ps_tr", name="ps_tr")
                    nc.tensor.transpose(pst[:m, :], la[:, :m], C["ident"][:])
                    sb = sp.tile([128, 128], F32, tag="ph_a2_sb", name="ph_a2_sb")
                    nc.scalar.copy(sb[:m, :], pst[:m, :])
                    nc.sync.dma_start(
                        xagg_full[t * 128:t * 128 + m, _ts(k, 128)], sb[:m, :])

            # =========== Phase B: GIN message aggregation (full graph) ======
            sgidx = cp.tile([128, D.EG_pad // 16], I16, name="sgidx_sb")
            nc.sync.dma_start(sgidx[:], I["SG"].ap())
            drel = cp.tile([128, D.EG_pad // 128], F32, name="drel_sb")
            nc.sync.dma_start(drel[:], I["DREL"].ap())

            tiles_per_win = D.PW // 128
            iota_bc = C["iota"][:].rearrange(
                "p (o f) -> p o f", o=1).broadcast_to(
                    (128, tiles_per_win, D.gwin))
            for w in range(D.NWIN if "b" in phases else 0):
                m = min(D.gwin, G - w * D.gwin)
                gath_g = wp.tile([128, tiles_per_win, H], F32,
                                 tag="ph_b_gath", name="ph_b_gath")
                nc.gpsimd.dma_gather(
                    gath_g[:], xagg_full[:],
                    sgidx[:, w * D.PW // 16:(w + 1) * D.PW // 16],
                    num_idxs=D.PW, num_idxs_reg=D.PW, elem_size=H)
                bhot = wp.tile([3 * D.VOCAB, D.PW], F32, tag="ph_b_bhot",
                               name="ph_b_bhot")
                nc.scalar.dma_start(bhot[:],
                                    I["BHOT"].ap()[:, _ts(w, D.PW)])
                # one-hot for the whole window in one op
                oh = sp.tile([128, tiles_per_win, D.gwin], F32, tag="ph_b_oh",
                             name="ph_b_oh")
                dr = drel[:, w * tiles_per_win:(w + 1) * tiles_per_win]
                nc.vector.tensor_tensor(
                    oh[:], iota_bc,
                    dr.rearrange("p (t o) -> p t o", o=1).broadcast_to(
                        (128, tiles_per_win, D.gwin)),
                    ALU.is_equal)
                # msg = relu(gathered + bond_emb), 2-tile psum groups
                msg = wp.tile([128, tiles_per_win, H], F32, tag="ph_b_msg",
                              name="ph_b_msg")
                for i0 in range(0, tiles_per_win, 2):
                    gn = min(2, tiles_per_win - i0)
                    ps_emb = pp.tile([128, 2, H], F32, tag="ps_w",
                                     name="ps_emb")
                    for j in range(gn):
                        nc.tensor.matmul(ps_emb[:, j, :],
                                         bhot[:, _ts(i0 + j, 128)],
                                         C["bondcat"][:], start=True,
                                         stop=True)
                    wbt = sp.tile([128, 2, H], F32, tag="ph_b_wbt",
                                  name="ph_b_wbt")
                    nc.vector.tensor_tensor(wbt[:, :gn, :],
                                            gath_g[:, i0:i0 + gn, :],
                                            ps_emb[:, :gn, :], ALU.add)
                    nc.scalar.activation(msg[:, i0:i0 + gn, :],
                                         wbt[:, :gn, :], ACTF.Relu)
                # feature-major agg via onehot matmuls: psT[k] [128, m]
                psT = [pp.tile([128, 128], F32, tag="ps_agg", name="ps_aggT0"),
                       pp.tile([128, 128], F32, tag="ps_mm", name="ps_aggT1")]
                for k in range(2):
                    for i in range(tiles_per_win):
                        nc.tensor.matmul(psT[k][:, :m],
                                         msg[:, i, _ts(k, 128)],
                                         oh[:, i, :m], start=(i == 0),
                                         stop=(i == tiles_per_win - 1))
                # tn[k] = (1+eps) * xaggT + agg_gT, straight to tT_d
                for k in range(2):
                    xa = sp.tile([128, 128], F32, tag="ph_b_xa", name="ph_b_xa")
                    nc.sync.dma_start(
                        xa[:, :m], xaggT_d[k][:, w * D.gwin:w * D.gwin + m])
                    tn = sp.tile([128, 128], F32, tag="ph_b_tn", name="ph_b_tn")
                    nc.vector.tensor_scalar(tn[:, :m], xa[:, :m],
                                            C["epsv"][:, :], None, ALU.mult)
                    nc.vector.tensor_tensor(tn[:, :m], tn[:, :m],
                                            psT[k][:, :m], ALU.add)
                    nc.sync.dma_start(
                        tT_d[k][:, w * D.gwin:w * D.gwin + m], tn[:, :m])

            # =========== GIN GEMM passes (streamed through DRAM) ============
            def bn_coeffs(st_acc, g_c, b_c, label):
                """scale/shift [128,1] per half from accumulated stats."""
                inv_n = 1.0 / float(G)
                out = []
                for k in range(2):
                    mu = sp.tile([128, 1], F32, tag=f"{label}_mu{k}",
                                 name=f"{label}_mu{k}")
                    nc.vector.tensor_scalar(mu[:], st_acc[:, 2 * k:2 * k + 1],
                                            inv_n, None, ALU.mult)
                    var = sp.tile([128, 1], F32, tag=f"{label}_va{k}",
                                  name=f"{label}_va{k}")
                    nc.vector.tensor_scalar(var[:],
                                            st_acc[:, 2 * k + 1:2 * k + 2],
                                            inv_n, None, ALU.mult)
                    mu2 = sp.tile([128, 1], F32, tag=f"{label}_m2{k}",
                                  name=f"{label}_m2{k}")
                    nc.vector.tensor_tensor(mu2[:], mu[:], mu[:], ALU.mult)
                    nc.vector.tensor_tensor(var[:], var[:], mu2[:],
                                            ALU.subtract)
                    sd = sp.tile([128, 1], F32, tag=f"{label}_sd{k}",
                                 name=f"{label}_sd{k}")
                    nc.scalar.activation(sd[:], var[:], ACTF.Sqrt,
                                         bias=C["eps5col"][:])
                    rs = sp.tile([128, 1], F32, tag=f"{label}_rs{k}",
                                 name=f"{label}_rs{k}")
                    nc.vector.reciprocal(rs[:], sd[:])
                    sc = cp.tile([128, 1], F32, name=f"{label}_sc{k}")
                    nc.vector.tensor_tensor(sc[:], g_c[k][:], rs[:], ALU.mult)
                    sh = cp.tile([128, 1], F32, name=f"{label}_sh{k}")
                    nc.vector.tensor_tensor(sh[:], mu[:], sc[:], ALU.mult)
                    nc.vector.tensor_tensor(sh[:], b_c[k][:], sh[:],
                                            ALU.subtract)
                    out.append((sc, sh))
                return out

            def gin_gemm(inT_d, Wc, outT_d, label, pre=None):
                """outT = W^T @ (pre(inT)) block-streamed, feature-major
                throughout (no transposes). Wc[k] is [128, H] = W rows
                [k*128:(k+1)*128, :]. Returns the stats accumulator."""
                st_acc = cp.tile([128, 4], F32, name=f"{label}_stacc")
                nc.vector.memset(st_acc[:], 0.0)
                BW = 512
                for b0 in range(0, G, BW):
                    bw = min(BW, G - b0)
                    tt = []
                    for k in range(2):
                        lt = wp.tile([128, BW], F32, tag=f"gmm_lt{k}",
                                     name=f"{label}_lt{k}")
                        nc.sync.dma_start(lt[:, :bw],
                                          inT_d[k][:, b0:b0 + bw])
                        if pre is not None:
                            sc, sh = pre[k]
                            nc.scalar.activation(lt[:, :bw], lt[:, :bw],
                                                 ACTF.Relu, bias=sh[:],
                                                 scale=sc[:])
                        tt.append(lt)
                    for fj in range(2):
                        ps = pp.tile([128, BW], F32, tag="ps_agg",
                                     name="ps_gmm")
                        for k in range(2):
                            nc.tensor.matmul(ps[:, :bw],
                                             Wc[k][:, _ts(fj, 128)],
                                             tt[k][:, :bw],
                                             start=(k == 0), stop=(k == 1))
                        ut = sp.tile([128, BW], F32, tag="gmm_ut",
                                     name=f"{label}_ut")
                        nc.scalar.copy(ut[:, :bw], ps[:, :bw])
                        nc.scalar.dma_start(
                            outT_d[fj][:, b0:b0 + bw], ut[:, :bw])
                        # stats accumulate
                        r1 = sp.tile([128, 1], F32, tag="gmm_r1",
                                     name=f"{label}_r1")
                        nc.vector.tensor_reduce(r1[:], ut[:, :bw], AX.X,
                                                ALU.add)
                        nc.vector.tensor_tensor(st_acc[:, 2 * fj:2 * fj + 1],
                                                st_acc[:, 2 * fj:2 * fj + 1],
                                                r1[:], ALU.add)
                        sq = sp.tile([128, BW], F32, tag="gmm_sq",
                                     name=f"{label}_sq")
                        nc.vector.tensor_tensor(sq[:, :bw], ut[:, :bw],
                                                ut[:, :bw], ALU.mult)
                        nc.vector.tensor_reduce(r1[:], sq[:, :bw], AX.X,
                                                ALU.add)
                        nc.vector.tensor_tensor(
                            st_acc[:, 2 * fj + 1:2 * fj + 2],
                            st_acc[:, 2 * fj + 1:2 * fj + 2],
                            r1[:], ALU.add)
                return st_acc

            if "b" in phases:
                st1 = gin_gemm(tT_d, C["gw1"], u1T_d, "gmm1")
                bn1 = bn_coeffs(st1, C["bn1g"], C["bn1b"], "bn1")
                st2 = gin_gemm(u1T_d, C["gw2"], u2T_d, "gmm2", pre=bn1)
                bn2 = bn_coeffs(st2, C["bn2g"], C["bn2b"], "bn2")
                # core's gin shard (first GS cols after rotation), bn2 applied
                ginT = []
                for k in range(2):
                    gt = cp.tile([128, GS], F32, name=f"ginT{k}")
                    gl = sp.tile([128, GS], F32, tag="gin_gl", name="gin_gl",
                                 bufs=1)
                    nc.sync.dma_start(gl[:], u2T_d[k][:, :GS])
                    sc, sh = bn2[k]
                    nc.scalar.activation(gt[:], gl[:], ACTF.Identity,
                                         bias=sh[:], scale=sc[:])
                    ginT.append(gt)
            else:
                ginT = []
                for k in range(2):
                    gt = cp.tile([128, GS], F32, name=f"ginT{k}")
                    nc.vector.memset(gt[:], 0.0)
                    ginT.append(gt)

            # =========== Phase C: conformer edge pipeline ===================
            # zero agg_dram
            zt = cp.tile([128, 512], F32, name="zero_sb")
            nc.vector.memset(zt[:], 0.0)
            zrows = 0
            NSg = NS + 128
            while zrows < NSg:
                r = min(512, NSg - zrows)
                p = 128 if r >= 128 else r
                r = (r // p) * p
                ap = agg_dram[zrows:zrows + r, :].rearrange(
                    "(t p) f -> p t f", p=p)
                zs = zt[:p, :r * NF // p].rearrange("p (t f) -> p t f", f=NF)
                nc.sync.dma_start(ap, zs)
                zrows += r

            # resident: C row (cosine cutoff per edge, tile layout)
            crow = cp.tile([128, D.E_pad // 128], F32, name="crow_sb")
            for s0 in range(0, D.E_pad // 128, 512):
                sw = min(512, D.E_pad // 128 - s0)
                wt = wp.tile([128, 512], F32, tag="ph_c_wt", name="ph_c_wt",
                             bufs=1)
                nc.sync.dma_start(wt[:, :sw], I["WT"].ap()[:, s0:s0 + sw])
                nc.scalar.activation(wt[:, :sw], wt[:, :sw], ACTF.Sin,
                                     bias=C["pihalf"][:],
                                     scale=math.pi / D.CUTOFF)
                nc.scalar.activation(crow[:, s0:s0 + sw], wt[:, :sw],
                                     ACTF.Copy, bias=0.5, scale=-0.5)

            NT = D.chunk // 128
            chunks_per_q = D.EQ_pad // D.chunk
            ph_bounds = list(np.cumsum(D.phase_sizes))
            HC = D.chunk // 2  # half-chunk = one gather call
            NTH = HC // 128
            b2v = C["b2full"][:].rearrange("p (t f) -> p t f", f=NF)
            SIW = 8 * D.chunk  # si/dsti block (edges)
            for q in range(D.NQ if "c" in phases else 0):
                qlo = q * D.qsize
                qe0 = q * D.EQ_pad
                si = dsti = None
                for cc in range(chunks_per_q):
                    e0 = qe0 + cc * D.chunk
                    le0 = cc * D.chunk  # quad-local edge offset
                    if cc % 8 == 0:
                        sb0 = le0
                        sbn = min(SIW, D.EQ_pad - sb0)
                        si = wp.tile([128, SIW // 16], I16, tag="ph_c_si",
                                     name="ph_c_si")
                        nc.sync.dma_start(
                            si[:, :sbn // 16],
                            I["SRC"].ap()[:, (qe0 + sb0) // 16:
                                          (qe0 + sb0 + sbn) // 16])
                        dsti = wp.tile([128, SIW // 16], I16, tag="ph_c_di",
                                       name="ph_c_di")
                        nc.sync.dma_start(
                            dsti[:, :sbn // 16],
                            I["DSTI"].ap()[:, (qe0 + sb0) // 16:
                                           (qe0 + sb0 + sbn) // 16])
                    so = le0 - sb0  # offset within si/dsti block
                    msg = wp.tile([128, NT, NF], F32, tag="ph_c_msg",
                                  name="ph_c_msg")
                    for hh in range(2):
                        h0 = hh * HC
                        gat = wp.tile([128, NTH, NF], F32, tag="ph_c_gat",
                                      name="ph_c_gat")
                        nc.gpsimd.dma_gather(
                            gat[:], xf_full[qlo:qlo + D.qsize, :],
                            si[:, (so + h0) // 16:(so + h0 + HC) // 16],
                            num_idxs=HC, num_idxs_reg=HC, elem_size=NF)
                        at = wp.tile([NG, HC], MMDT, tag="ph_c_at",
                                     name="ph_c_at")
                        nc.sync.dma_start(
                            at[:], I["AT"].ap()[:, e0 + h0:e0 + h0 + HC])
                        h1 = wp.tile([128, HC], MMDT, tag="ph_c_h1",
                                     name="ph_c_h1")
                        for s0 in range(0, HC, 512):
                            sw = min(512, HC - s0)
                            ps1 = pp.tile([128, 512], F32, tag="ps_mm",
                                          name="ps_mm")
                            nc.tensor.matmul(ps1[:, :sw], C["w1"][:],
                                             at[:, s0:s0 + sw], start=True,
                                             stop=True)
                            nc.scalar.activation(h1[:, s0:s0 + sw], ps1[:, :sw],
                                                 ACTF.Relu, bias=C["b1col"][:])
                        for g0 in range(0, NTH, 4):
                            psw = pp.tile([128, 4, NF], F32, tag="ps_w",
                                          name="ps_w")
                            for ti in range(4):
                                t = g0 + ti
                                nc.tensor.matmul(psw[:, ti, :],
                                                 h1[:, _ts(t, 128)],
                                                 C["w2"][:], start=True,
                                                 stop=True)
                            wb = sp.tile([128, 4, NF], F32, tag="ph_c_wb",
                                         name="ph_c_wb")
                            nc.vector.tensor_tensor(wb[:], psw[:], b2v,
                                                    ALU.add)
                            cb = (e0 + h0) // 128 + g0
                            cr = crow[:, cb:cb + 4]
                            nc.vector.tensor_tensor(
                                wb[:], wb[:], cr.broadcast_to((128, 4, NF)),
                                ALU.mult)
                            nc.vector.tensor_tensor(
                                msg[:, hh * NTH + g0:hh * NTH + g0 + 4, :],
                                wb[:], gat[:, g0:g0 + 4, :], ALU.mult)
                    # scatter-add into agg: split calls at phase boundaries
                    c0 = cc * D.chunk
                    c1 = c0 + D.chunk
                    cuts = {c0, c1}
                    for b in ph_bounds:
                        if c0 < b < c1:
                            cuts.add(int(b))
                    cuts = sorted(cuts)
                    for a, b in zip(cuts[:-1], cuts[1:]):
                        for s in range(a, b, D.dcall):
                            sn = min(D.dcall, b - s)
                            la = s - c0
                            nc.gpsimd.dma_scatter_add(
                                agg_dram[:],
                                msg[:, la // 128:(la + sn) // 128, :],
                                dsti[:, (so + la) // 16:(so + la + sn) // 16],
                                num_idxs=sn, num_idxs_reg=sn, elem_size=NF)

            # =========== Phase D: h = relu(agg@lin2+b)@linw+b, residual =====
            NCH = D.nchunk
            n_nch = NS // NCH
            for j in range(n_nch):
                r0 = j * NCH
                aggT = wp.tile([NF, NCH], F32, tag="ph_d_aggT", name="ph_d_aggT")
                PB = 125  # NCH = 4 * PB, rows wrapped 125/partition
                asb = wp.tile([PB, 4, NF], F32, tag="ph_d_asb", name="ph_d_asb")
                nc.sync.dma_start(
                    asb[:],
                    agg_dram[r0:r0 + NCH, :].rearrange("(t p) f -> p t f",
                                                       p=PB))
                for t in range(4):
                    pst = pp.tile([128, 128], F32, tag="ps_tr", name="ps_tr")
                    nc.tensor.transpose(pst[:, :PB], asb[:PB, t, :],
                                        C["ident"][:PB, :PB])
                    nc.vector.tensor_copy(aggT[:, t * PB:(t + 1) * PB],
                                          pst[:, :PB])
                h1T = [wp.tile([128, NCH], F32, tag=f"ph_d_h1T{k}",
                               name=f"ph_d_h1T{k}")
                       for k in range(2)]
                for k in range(2):
                    ps = pp.tile([128, NCH], F32, tag="ps_mm", name="ps_mm")
                    nc.tensor.matmul(ps[:], C["lin2"][:, _ts(k, 128)], aggT[:],
                                     start=True, stop=True)
                    nc.scalar.activation(h1T[k][:], ps[:], ACTF.Relu,
                                         bias=C["lin2b"][k][:])
                for k in range(2):
                    ps = pp.tile([128, NCH], F32, tag="ps_mm", name="ps_mm")
                    for kk in range(2):
                        nc.tensor.matmul(ps[:], C["linw"][kk][:, _ts(k, 128)],
                                         h1T[kk][:], start=(kk == 0),
                                         stop=(kk == 1))
                    ob = sp.tile([128, NCH], F32, tag="ph_d_ob", name="ph_d_ob")
                    nc.scalar.activation(ob[:], ps[:], ACTF.Identity,
                                         bias=C["linb"][k][:])
                    xtc = sp.tile([128, NCH], F32, tag="ph_d_xtc", name="ph_d_xtc")
                    nc.sync.dma_start(xtc[:], I["xT"].ap()[k, :, r0:r0 + NCH])
                    nc.vector.tensor_tensor(ob[:], ob[:], xtc[:], ALU.add)
                    g0 = r0 // rep
                    gin_rep = ginT[k][:, g0:g0 + NCH // rep].broadcast_to(
                        (128, NCH // rep, rep))
                    nc.vector.tensor_tensor(
                        ob[:].rearrange("p (g t) -> p g t", t=rep),
                        ob[:].rearrange("p (g t) -> p g t", t=rep),
                        gin_rep, ALU.add)
                    nc.sync.dma_start(outT.ap()[k, :, r0:r0 + NCH], ob[:])

    nc.compile()
    return nc


_CACHE = {}


def _get_nc(D: Dims, phases: str = "abcd"):
    key = ("nc", D, phases)
    if key not in _CACHE:
        _CACHE[key] = build_nc(D, phases)
    return _CACHE[key]


def run_on_hw(inputs, D: Dims = REAL):
    nc = _get_nc(D)
    in_maps = host_prep(inputs, D)
    res = bass_utils.run_bass_kernel_spmd(nc, in_maps,
                                          core_ids=list(range(D.cores)))
    return assemble(res.results, D)


def kernel(**inputs):
    return run_on_hw(inputs, REAL)


# revision 4
# speedup vs baseline: 1.1562x; 1.1562x over previous
"""Trainium2 Bass kernel for nn_DSSConf — v2: zero collectives.

Design: replicate the full x to every core, ROTATED by the core's node
offset so the SPMD program is fully static (each core's own shard is
always at local offset 0). Each core computes the full xf table and the
full (replicated) GIN branch locally, then processes its 1/8 of the
conformer edges and emits its output shard. No collectives at all.

Self-contained: hardcodes shapes/sharding; exposes kernel(**inputs).
"""
import sys
import math
from dataclasses import dataclass

sys.path.insert(0, "/opt/trn_rl_repo")

import numpy as np
from concourse import bass, bacc, tile, mybir, bass_utils

F32 = mybir.dt.float32
BF16 = mybir.dt.bfloat16
I16 = mybir.dt.int16
ALU = mybir.AluOpType
ACTF = mybir.ActivationFunctionType
AX = mybir.AxisListType

WMLP_BF16 = False  # edge-filter MLP matmuls in bf16 (flip after precision exp)
GIN_BF16 = False


@dataclass(frozen=True)
class Dims:
    N: int = 100000        # conformer nodes
    H: int = 256           # hidden
    NF: int = 128          # num filters
    NG: int = 50           # num gaussians
    G: int = 10000         # graph nodes
    E: int = 1000000       # conformer edges
    EG: int = 30000        # graph edges
    VOCAB: int = 5
    CUTOFF: float = 10.0
    cores: int = 8
    qsize: int = 25000     # src quadrant size for int16 gather indices
    chunk: int = 2048      # conformer edge chunk (multiple of 128, divides EQ_pad)
    dcall: int = 1024      # max descriptors per gather/scatter DMA call
                           # (SWDGE ring = dynamic_dma_scratch_size/16 = 1024)
    gwin: int = 128        # GIN scatter window (<=128 segments)
    PW: int = 512          # padded GIN edges per window (multiple of 128)
    GW: int = 1            # GIN windows per gather call
    nchunk: int = 500      # node chunk for the h/out stage (divides NS, mult of 10)
    pch: int = 1280        # Phase A node chunk (mult of 128 and of rep=10)

    @property
    def NS(self):
        return self.N // self.cores

    @property
    def GS(self):
        return self.G // self.cores

    @property
    def NQ(self):
        return (self.N + self.qsize - 1) // self.qsize

    @property
    def phase_sizes(self):
        """Fixed per-quadrant edge-bucket sizes, one per dst-occurrence
        rank. Within a bucket every dst is unique -> dma_scatter_add calls
        that stay inside a bucket are race-free. Sized from a Poisson model
        with margin; host_prep asserts the actual counts fit."""
        lam = (self.E / (self.cores * self.NQ)) / self.NS
        sizes = []
        pmf = math.exp(-lam)
        cdf = pmf
        p = 0
        while True:
            sf = 1.0 - cdf  # P(X >= p+1)
            mp = self.NS * sf
            if mp < 0.02 and p > 0:
                break
            pad = int(mp + 4.0 * math.sqrt(max(mp, 1.0)) + 64)
            sizes.append((pad + 127) // 128 * 128)
            p += 1
            pmf *= lam / p
            cdf += pmf
            if p > 64:
                break
        tot = sum(sizes)
        eq = (tot + self.chunk - 1) // self.chunk * self.chunk
        sizes[-1] += eq - tot
        return sizes

    @property
    def EQ_pad(self):
        return sum(self.phase_sizes)

    @property
    def E_pad(self):
        return self.NQ * self.EQ_pad

    @property
    def NWIN(self):
        return (self.G + self.gwin - 1) // self.gwin

    @property
    def EG_pad(self):
        return self.NWIN * self.PW


REAL = Dims()


def _wrap16(arr, dtype=np.int16):
    """Edge i -> [i % 16, i // 16], replicated to 128 partitions."""
    a = np.asarray(arr).reshape(-1, 16).T.astype(dtype)
    return np.tile(a, (8, 1)).copy()


def _tile128(arr, dtype=np.float32):
    """Edge i -> [i % 128, i // 128] (per-partition scalar layout)."""
    return np.ascontiguousarray(np.asarray(arr).reshape(-1, 128).T.astype(dtype))


def host_prep(inputs, D: Dims):
    """Build per-core in_maps (list of dicts) for the SPMD kernel."""
    x = np.asarray(inputs["x"], np.float32)
    cnb = np.asarray(inputs["conf_node_batch"]).astype(np.int64)
    ei = np.asarray(inputs["edge_index_conf"]).astype(np.int64)
    ew = np.asarray(inputs["edge_weight_conf"], np.float32)
    ea = np.asarray(inputs["edge_attr_conf"], np.float32)
    eig = np.asarray(inputs["edge_index_graph"]).astype(np.int64)
    eag = np.asarray(inputs["edge_attr_graph"]).astype(np.int64)

    rep = D.N // D.G
    assert np.array_equal(cnb, np.repeat(np.arange(D.G), rep)), \
        "conf_node_batch structure mismatch"

    NS, GS = D.NS, D.GS
    src, dst = ei[0], ei[1]
    owner = dst // NS
    sizes = D.phase_sizes
    ph_off = np.concatenate([[0], np.cumsum(sizes)])[:-1]

    xT = np.ascontiguousarray(x.T).reshape(2, 128, D.N)

    # GIN edges (global)
    sg, dg = eig[0], eig[1]

    # ---- weights (replicated) ----
    H2 = D.H // 128
    w = {k: np.asarray(inputs[k], np.float32) for k in (
        "mlp_w1", "mlp_b1", "mlp_w2", "mlp_b2", "cf_lin1", "cf_lin2",
        "cf_lin2_b", "lin_w", "lin_b", "bond_emb", "gin_eps", "gin_w1",
        "gin_w2", "bn1_g", "bn1_b", "bn2_g", "bn2_b")}
    mmdt = np.float32
    const = dict(
        w1=w["mlp_w1"].astype(mmdt),                      # [NG, NF]
        b1col=w["mlp_b1"].reshape(D.NF, 1),
        w2=w["mlp_w2"].astype(mmdt),                      # [NF, NF]
        b2full=np.tile(w["mlp_b2"].reshape(1, D.NF), (128, 4)).copy(),
        lin1=np.ascontiguousarray(w["cf_lin1"].reshape(H2, 128, D.NF)),
        lin2=w["cf_lin2"],                                # [NF, H]
        lin2b=w["cf_lin2_b"].reshape(H2, 128, 1),
        linw=np.ascontiguousarray(w["lin_w"].reshape(H2, 128, D.H)),
        linb=w["lin_b"].reshape(H2, 128, 1),
        gw1=np.ascontiguousarray(w["gin_w1"].reshape(H2, 128, D.H)),
        gw2=np.ascontiguousarray(w["gin_w2"].reshape(H2, 128, D.H)),
        bondcat=np.ascontiguousarray(
            w["bond_emb"].reshape(3 * D.VOCAB, D.H)),
        bn1g=w["bn1_g"].reshape(H2, 128, 1), bn1b=w["bn1_b"].reshape(H2, 128, 1),
        bn2g=w["bn2_g"].reshape(H2, 128, 1), bn2b=w["bn2_b"].reshape(H2, 128, 1),
        epsv=np.full((128, 1), 1.0 + float(w["gin_eps"]), np.float32),
        zerocol=np.zeros((128, 1), np.float32),
        eps5col=np.full((128, 1), 1e-5, np.float32),
        pihalf=np.full((128, 1), -math.pi / 2, np.float32),
        iota=np.tile(np.arange(128, dtype=np.float32), (128, 1)).copy(),
        ident=np.eye(128, dtype=np.float32),
    )

    in_maps = []
    for c in range(D.cores):
        # ---- rotated full x ----
        off = c * NS
        xr = np.concatenate([xT[:, :, off:], xT[:, :, :off]], axis=2)

        # ---- conformer edges owned by this core ----
        sel = owner == c
        s = src[sel]
        dd = dst[sel] - off                      # [0, NS)
        sr = (s - off) % D.N                     # rotated src
        q = sr // D.qsize
        order = np.lexsort((dd, q))
        s_sr, s_d, s_q = sr[order], dd[order], q[order]
        s_w = ew[sel][order]
        s_a = ea[sel][order]

        src_pad = np.zeros(D.E_pad, np.int64)
        dst_pad = NS + (np.arange(D.E_pad, dtype=np.int64) % 128)
        w_pad = np.full(D.E_pad, D.CUTOFF, np.float32)   # C(CUTOFF)=0 kills pads
        a_pad = np.zeros((D.E_pad, D.NG), np.float32)
        bounds = np.searchsorted(s_q, np.arange(D.NQ + 1))
        for qq in range(D.NQ):
            lo, hi = bounds[qq], bounds[qq + 1]
            d = s_d[lo:hi]                        # dst-sorted within the bucket
            rank = np.arange(len(d)) - np.searchsorted(d, d)
            counts = np.bincount(rank, minlength=len(sizes))
            assert len(counts) <= len(sizes) and (counts <= sizes).all(), \
                f"phase overflow: core {c} quad {qq}: {counts} vs {sizes}"
            o = qq * D.EQ_pad
            pos = np.empty(len(d), np.int64)
            for p in np.unique(rank):
                psel = rank == p
                pos[psel] = o + ph_off[p] + np.arange(counts[p])
            src_pad[pos] = s_sr[lo:hi] - qq * D.qsize
            dst_pad[pos] = d
            w_pad[pos] = s_w[lo:hi]
            a_pad[pos] = s_a[lo:hi]

        # ---- GIN edges (full graph, rotated) ----
        sgr = (sg - c * GS) % D.G
        dgr = (dg - c * GS) % D.G
        gw_ = dgr // D.gwin
        gorder = np.lexsort((dgr, gw_))
        g_s, g_d, g_w = sgr[gorder], dgr[gorder], gw_[gorder]
        g_a = eag[gorder]

        sg_pad = np.zeros(D.EG_pad, np.int64)
        dr_pad = np.full(D.EG_pad, -1.0, np.float32)  # -1 kills pads in one-hot
        bh_pad = np.zeros((3 * D.VOCAB, D.EG_pad), np.float32)
        gbounds = np.searchsorted(g_w, np.arange(D.NWIN + 1))
        for ww in range(D.NWIN):
            lo, hi = gbounds[ww], gbounds[ww + 1]
            cnt = hi - lo
            assert cnt <= D.PW, f"PW overflow: core {c} win {ww}: {cnt}"
            o = ww * D.PW
            sg_pad[o:o + cnt] = g_s[lo:hi]
            dr_pad[o:o + cnt] = (g_d[lo:hi] - ww * D.gwin).astype(np.float32)
            for k in range(3):
                bh_pad[k * D.VOCAB + g_a[lo:hi, k], np.arange(o, o + cnt)] = 1.0

        m = dict(
            xT=np.ascontiguousarray(xr),
            AT=np.ascontiguousarray(a_pad.T),
            WT=_tile128(w_pad),
            SRC=_wrap16(src_pad),
            DSTI=_wrap16(dst_pad),
            SG=_wrap16(sg_pad),
            DREL=_tile128(dr_pad),
            BHOT=bh_pad,
        )
        m.update(const)
        in_maps.append(m)
    return in_maps


def assemble(results, D: Dims):
    """Per-core outT [2,128,NS] -> full [N, H]."""
    parts = [r["outT"].reshape(D.H, D.NS) for r in results]
    outT = np.concatenate(parts, axis=1)  # [H, N]
    return np.ascontiguousarray(outT.T)


def _ts(i, n):
    return bass.ts(i, n)


def build_nc(D: Dims, phases: str = "abcd"):
    nc = bacc.Bacc("TRN2", target_bir_lowering=False, debug=False,
                   num_devices=D.cores)
    NS, GS, H, NF, NG, G = D.NS, D.GS, D.H, D.NF, D.NG, D.G
    H2 = H // 128
    rep = D.N // D.G
    MMDT = F32

    I = {}
    def di(name, shape, dt=F32):
        I[name] = nc.dram_tensor(name, list(shape), dt, kind="ExternalInput")
        return I[name]

    di("xT", [2, 128, D.N])
    di("AT", [NG, D.E_pad])
    di("WT", [128, D.E_pad // 128])
    di("SRC", [128, D.E_pad // 16], I16)
    di("DSTI", [128, D.E_pad // 16], I16)
    di("SG", [128, D.EG_pad // 16], I16)
    di("DREL", [128, D.EG_pad // 128])
    di("BHOT", [3 * D.VOCAB, D.EG_pad])
    di("w1", [NG, NF], MMDT); di("b1col", [NF, 1]); di("w2", [NF, NF], MMDT)
    di("b2full", [128, 4 * NF])
    di("lin1", [H2, 128, NF]); di("lin2", [NF, H]); di("lin2b", [H2, 128, 1])
    di("linw", [H2, 128, H]); di("linb", [H2, 128, 1])
    di("gw1", [H2, 128, H]); di("gw2", [H2, 128, H])
    di("bondcat", [3 * D.VOCAB, H])
    di("bn1g", [H2, 128, 1]); di("bn1b", [H2, 128, 1])
    di("bn2g", [H2, 128, 1]); di("bn2b", [H2, 128, 1])
    di("epsv", [128, 1]); di("iota", [128, 128]); di("ident", [128, 128])
    di("zerocol", [128, 1]); di("eps5col", [128, 1]); di("pihalf", [128, 1])

    outT = nc.dram_tensor("outT", [2, 128, NS], F32, kind="ExternalOutput")

    with tile.TileContext(nc) as tc:
        with (
            tc.tile_pool(name="const", bufs=1) as cp,
            tc.tile_pool(name="work", bufs=2) as wp,
            tc.tile_pool(name="small", bufs=2) as sp,
            tc.tile_pool(name="psum", bufs=2, space="PSUM") as pp,
            tc.tile_pool(name="dram", bufs=1, space="DRAM") as dp,
        ):
            # ---------- load constants ----------
            C = {}
            for nm, shp, dt in [("w1", [NG, NF], MMDT), ("b1col", [NF, 1], F32),
                                ("w2", [NF, NF], MMDT),
                                ("b2full", [128, 4 * NF], F32),
                                ("lin2", [NF, H], F32),
                                ("bondcat", [3 * D.VOCAB, H], F32),
                                ("epsv", [128, 1], F32), ("iota", [128, 128], F32),
                                ("ident", [128, 128], F32),
                                ("zerocol", [128, 1], F32),
                                ("eps5col", [128, 1], F32),
                                ("pihalf", [128, 1], F32)]:
                t = cp.tile(shp, dt, name=f"c_{nm}")
                nc.sync.dma_start(t[:], I[nm].ap())
                C[nm] = t
            nc.const_aps.aps[(F32, 0.0)] = C["zerocol"][:]
            for nm in ("lin1", "lin2b", "linw", "linb", "gw1", "gw2",
                       "bn1g", "bn1b", "bn2g", "bn2b"):
                C[nm] = []
                inner = I[nm].shape[2]
                for k in range(H2):
                    t = cp.tile([128, inner], F32, name=f"c_{nm}{k}")
                    nc.sync.dma_start(t[:], I[nm].ap()[k])
                    C[nm].append(t)

            # ---------- DRAM scratch (all core-local) ----------
            xf_full = dp.tile([D.N, NF], F32, name="xf_full")
            xaggT_d = dp.tile([2, 128, G], F32, name="xaggT_d")
            xagg_full = dp.tile([G, H], F32, name="xagg_full")
            tT_d = dp.tile([2, 128, G], F32, name="tT_d")
            u1T_d = dp.tile([2, 128, G], F32, name="u1T_d")
            u2T_d = dp.tile([2, 128, G], F32, name="u2T_d")
            agg_dram = dp.tile([NS + 128, NF], F32, name="agg_dram")

            # =========== Phase A: xf for ALL nodes + segment-max pool =======
            # Full chunks of PCH (mult of 128 and rep), ragged tail after.
            PCH = D.pch
            n_full = D.N // PCH if "a" in phases else 0
            NTA = PCH // 128
            for j in range(n_full):
                xt = [wp.tile([128, PCH], F32, tag=f"ph_a_xt{k}",
                              name=f"ph_a_xt{k}")
                      for k in range(2)]
                for k in range(2):
                    nc.sync.dma_start(xt[k][:], I["xT"].ap()[k, :, _ts(j, PCH)])
                # pool: max over groups of `rep` cols -> xaggT_d chunk cols
                for k in range(2):
                    xa_sb = sp.tile([128, PCH // rep], F32, tag="ph_a_poolsb",
                                    name="ph_a_poolsb")
                    nc.vector.tensor_reduce(
                        xa_sb[:],
                        xt[k][:].rearrange("p (g t) -> p g t", t=rep),
                        AX.X, ALU.max)
                    nc.sync.dma_start(
                        xaggT_d[k][:, _ts(j, PCH // rep)], xa_sb[:])
                # xf = x @ cf_lin1: mm pairs into 4-tile psum banks, one
                # bulk row-wrapped write per chunk
                sb = wp.tile([128, NTA, NF], F32, tag="ph_a_sb", name="ph_a_sb")
                for g0 in range(0, NTA, 4):
                    gn = min(4, NTA - g0)
                    ps = pp.tile([128, 4, NF], F32, tag="ps_mm", name="ps_mm")
                    for ti in range(gn):
                        t = g0 + ti
                        for k in range(2):
                            nc.tensor.matmul(ps[:, ti, :],
                                             xt[k][:, t * 128:(t + 1) * 128],
                                             C["lin1"][k][:], start=(k == 0),
                                             stop=(k == 1))
                    nc.scalar.copy(sb[:, g0:g0 + gn, :], ps[:, :gn, :])
                nc.scalar.dma_start(
                    xf_full[j * PCH:(j + 1) * PCH, :].rearrange(
                        "(t p) f -> p t f", p=128),
                    sb[:])
            # ragged tail (node-at-a-time tiles)
            tail0 = n_full * PCH
            n_tail = D.N - tail0 if "a" in phases else 0
            if n_tail:
                xt = [wp.tile([128, n_tail], F32, tag=f"ph_a_xt{k}",
                              name=f"ph_a_xtt{k}")
                      for k in range(2)]
                for k in range(2):
                    nc.sync.dma_start(xt[k][:],
                                      I["xT"].ap()[k, :, tail0:tail0 + n_tail])
                for k in range(2):
                    xa_sb = sp.tile([128, n_tail // rep], F32,
                                    tag="ph_a_poolsb", name="ph_a_poolsbt")
                    nc.vector.tensor_reduce(
                        xa_sb[:],
                        xt[k][:].rearrange("p (g t) -> p g t", t=rep),
                        AX.X, ALU.max)
                    nc.sync.dma_start(
                        xaggT_d[k][:, tail0 // rep:(tail0 + n_tail) // rep],
                        xa_sb[:])
                for t in range((n_tail + 127) // 128):
                    m = min(128, n_tail - t * 128)
                    ps = pp.tile([128, 4, NF], F32, tag="ps_mm", name="ps_mm")
                    for k in range(2):
                        nc.tensor.matmul(ps[:m, 0, :],
                                         xt[k][:, t * 128:t * 128 + m],
                                         C["lin1"][k][:], start=(k == 0),
                                         stop=(k == 1))
                    sb = sp.tile([128, NF], F32, tag="ph_a_tsb", name="ph_a_tsb")
                    nc.scalar.copy(sb[:m, :], ps[:m, 0, :])
                    nc.sync.dma_start(
                        xf_full[tail0 + t * 128: tail0 + t * 128 + m, :],
                        sb[:m, :])

            # =========== Phase A2: transpose xagg -> node-major =============
            GT = (G + 127) // 128
            for t in range(GT if "a" in phases else 0):
                m = min(128, G - t * 128)
                for k in range(2):
                    la = sp.tile([128, 128], F32, tag="ph_a2_la", name="ph_a2_la")
                    nc.scalar.dma_start(la[:, :m],
                                        xaggT_d[k][:, t * 128:t * 128 + m])
                    pst = pp.tile([128, 128], F32, tag="ps_tr", name="ps_tr")
                    nc.tensor.transpose(pst[:m, :], la[:, :m], C["ident"][:])
                    sb = sp.tile([128, 128], F32, tag="ph_a2_sb", name="ph_a2_sb")
                    nc.scalar.copy(sb[:m, :], pst[:m, :])
                    nc.sync.dma_start(
                        xagg_full[t * 128:t * 128 + m, _ts(k, 128)], sb[:m, :])

            # =========== Phase B: GIN message aggregation (full graph) ======
            sgidx = cp.tile([128, D.EG_pad // 16], I16, name="sgidx_sb")
            nc.sync.dma_start(sgidx[:], I["SG"].ap())
            drel = cp.tile([128, D.EG_pad // 128], F32, name="drel_sb")
            nc.sync.dma_start(drel[:], I["DREL"].ap())

            tiles_per_win = D.PW // 128
            iota_bc = C["iota"][:].rearrange(
                "p (o f) -> p o f", o=1).broadcast_to(
                    (128, tiles_per_win, D.gwin))
            for w in range(D.NWIN if "b" in phases else 0):
                m = min(D.gwin, G - w * D.gwin)
                gath_g = wp.tile([128, tiles_per_win, H], F32,
                                 tag="ph_b_gath", name="ph_b_gath")
                nc.gpsimd.dma_gather(
                    gath_g[:], xagg_full[:],
                    sgidx[:, w * D.PW // 16:(w + 1) * D.PW // 16],
                    num_idxs=D.PW, num_idxs_reg=D.PW, elem_size=H)
                bhot = wp.tile([3 * D.VOCAB, D.PW], F32, tag="ph_b_bhot",
                               name="ph_b_bhot")
                nc.scalar.dma_start(bhot[:],
                                    I["BHOT"].ap()[:, _ts(w, D.PW)])
                # one-hot for the whole window in one op
                oh = sp.tile([128, tiles_per_win, D.gwin], F32, tag="ph_b_oh",
                             name="ph_b_oh")
                dr = drel[:, w * tiles_per_win:(w + 1) * tiles_per_win]
                nc.vector.tensor_tensor(
                    oh[:], iota_bc,
                    dr.rearrange("p (t o) -> p t o", o=1).broadcast_to(
                        (128, tiles_per_win, D.gwin)),
                    ALU.is_equal)
                # msg = relu(gathered + bond_emb), 2-tile psum groups
                msg = wp.tile([128, tiles_per_win, H], F32, tag="ph_b_msg",
                              name="ph_b_msg")
                for i0 in range(0, tiles_per_win, 2):
                    gn = min(2, tiles_per_win - i0)
                    ps_emb = pp.tile([128, 2, H], F32, tag="ps_w",
                                     name="ps_emb")
                    for j in range(gn):
                        nc.tensor.matmul(ps_emb[:, j, :],
                                         bhot[:, _ts(i0 + j, 128)],
                                         C["bondcat"][:], start=True,
                                         stop=True)
                    wbt = sp.tile([128, 2, H], F32, tag="ph_b_wbt",
                                  name="ph_b_wbt")
                    nc.vector.tensor_tensor(wbt[:, :gn, :],
                                            gath_g[:, i0:i0 + gn, :],
                                            ps_emb[:, :gn, :], ALU.add)
                    nc.scalar.activation(msg[:, i0:i0 + gn, :],
                                         wbt[:, :gn, :], ACTF.Relu)
                # feature-major agg via onehot matmuls: psT[k] [128, m]
                psT = [pp.tile([128, 128], F32, tag="ps_agg", name="ps_aggT0"),
                       pp.tile([128, 128], F32, tag="ps_mm", name="ps_aggT1")]
                for k in range(2):
                    for i in range(tiles_per_win):
                        nc.tensor.matmul(psT[k][:, :m],
                                         msg[:, i, _ts(k, 128)],
                                         oh[:, i, :m], start=(i == 0),
                                         stop=(i == tiles_per_win - 1))
                # tn[k] = (1+eps) * xaggT + agg_gT, straight to tT_d
                for k in range(2):
                    xa = sp.tile([128, 128], F32, tag="ph_b_xa", name="ph_b_xa")
                    nc.sync.dma_start(
                        xa[:, :m], xaggT_d[k][:, w * D.gwin:w * D.gwin + m])
                    tn = sp.tile([128, 128], F32, tag="ph_b_tn", name="ph_b_tn")
                    nc.vector.tensor_scalar(tn[:, :m], xa[:, :m],
                                            C["epsv"][:, :], None, ALU.mult)
                    nc.vector.tensor_tensor(tn[:, :m], tn[:, :m],
                                            psT[k][:, :m], ALU.add)
                    nc.sync.dma_start(
                        tT_d[k][:, w * D.gwin:w * D.gwin + m], tn[:, :m])

            # =========== GIN GEMM passes (streamed through DRAM) ============
            def bn_coeffs(st_acc, g_c, b_c, label):
                """scale/shift [128,1] per half from accumulated stats."""
                inv_n = 1.0 / float(G)
                out = []
                for k in range(2):
                    mu = sp.tile([128, 1], F32, tag=f"{label}_mu{k}",
                                 name=f"{label}_mu{k}")
                    nc.vector.tensor_scalar(mu[:], st_acc[:, 2 * k:2 * k + 1],
                                            inv_n, None, ALU.mult)
                    var = sp.tile([128, 1], F32, tag=f"{label}_va{k}",
                                  name=f"{label}_va{k}")
                    nc.vector.tensor_scalar(var[:],
                                            st_acc[:, 2 * k + 1:2 * k + 2],
                                            inv_n, None, ALU.mult)
                    mu2 = sp.tile([128, 1], F32, tag=f"{label}_m2{k}",
                                  name=f"{label}_m2{k}")
                    nc.vector.tensor_tensor(mu2[:], mu[:], mu[:], ALU.mult)
                    nc.vector.tensor_tensor(var[:], var[:], mu2[:],
                                            ALU.subtract)
                    sd = sp.tile([128, 1], F32, tag=f"{label}_sd{k}",
                                 name=f"{label}_sd{k}")
                    nc.scalar.activation(sd[:], var[:], ACTF.Sqrt,
                                         bias=C["eps5col"][:])
                    rs = sp.tile([128, 1], F32, tag=f"{label}_rs{k}",
                                 name=f"{label}_rs{k}")
                    nc.vector.reciprocal(rs[:], sd[:])
                    sc = cp.tile([128, 1], F32, name=f"{label}_sc{k}")
                    nc.vector.tensor_tensor(sc[:], g_c[k][:], rs[:], ALU.mult)
                    sh = cp.tile([128, 1], F32, name=f"{label}_sh{k}")
                    nc.vector.tensor_tensor(sh[:], mu[:], sc[:], ALU.mult)
                    nc.vector.tensor_tensor(sh[:], b_c[k][:], sh[:],
                                            ALU.subtract)
                    out.append((sc, sh))
                return out

            def gin_gemm(inT_d, Wc, outT_d, label, pre=None):
                """outT = W^T @ (pre(inT)) block-streamed, feature-major
                throughout (no transposes). Wc[k] is [128, H] = W rows
                [k*128:(k+1)*128, :]. Returns the stats accumulator."""
                st_acc = cp.tile([128, 4], F32, name=f"{label}_stacc")
                nc.vector.memset(st_acc[:], 0.0)
                BW = 512
                for b0 in range(0, G, BW):
                    bw = min(BW, G - b0)
                    tt = []
                    for k in range(2):
                        lt = wp.tile([128, BW], F32, tag=f"gmm_lt{k}",
                                     name=f"{label}_lt{k}")
                        nc.sync.dma_start(lt[:, :bw],
                                          inT_d[k][:, b0:b0 + bw])
                        if pre is not None:
                            sc, sh = pre[k]
                            nc.scalar.activation(lt[:, :bw], lt[:, :bw],
                                                 ACTF.Relu, bias=sh[:],
                                                 scale=sc[:])
                        tt.append(lt)
                    for fj in range(2):
                        ps = pp.tile([128, BW], F32, tag="ps_agg",
                                     name="ps_gmm")
                        for k in range(2):
                            nc.tensor.matmul(ps[:, :bw],
                                             Wc[k][:, _ts(fj, 128)],
                                             tt[k][:, :bw],
                                             start=(k == 0), stop=(k == 1))
                        ut = sp.tile([128, BW], F32, tag="gmm_ut",
                                     name=f"{label}_ut")
                        nc.scalar.copy(ut[:, :bw], ps[:, :bw])
                        nc.scalar.dma_start(
                            outT_d[fj][:, b0:b0 + bw], ut[:, :bw])
                        # stats accumulate
                        r1 = sp.tile([128, 1], F32, tag="gmm_r1",
                                     name=f"{label}_r1")
                        nc.vector.tensor_reduce(r1[:], ut[:, :bw], AX.X,
                                                ALU.add)
                        nc.vector.tensor_tensor(st_acc[:, 2 * fj:2 * fj + 1],
                                                st_acc[:, 2 * fj:2 * fj + 1],
                                                r1[:], ALU.add)
                        sq = sp.tile([128, BW], F32, tag="gmm_sq",
                                     name=f"{label}_sq")
                        nc.vector.tensor_tensor(sq[:, :bw], ut[:, :bw],
                                                ut[:, :bw], ALU.mult)
                        nc.vector.tensor_reduce(r1[:], sq[:, :bw], AX.X,
                                                ALU.add)
                        nc.vector.tensor_tensor(
                            st_acc[:, 2 * fj + 1:2 * fj + 2],
                            st_acc[:, 2 * fj + 1:2 * fj + 2],
                            r1[:], ALU.add)
                return st_acc

            if "b" in phases:
                st1 = gin_gemm(tT_d, C["gw1"], u1T_d, "gmm1")
                bn1 = bn_coeffs(st1, C["bn1g"], C["bn1b"], "bn1")
                st2 = gin_gemm(u1T_d, C["gw2"], u2T_d, "gmm2", pre=bn1)
                bn2 = bn_coeffs(st2, C["bn2g"], C["bn2b"], "bn2")
                # core's gin shard (first GS cols after rotation), bn2 applied
                ginT = []
                for k in range(2):
                    gt = cp.tile([128, GS], F32, name=f"ginT{k}")
                    gl = sp.tile([128, GS], F32, tag="gin_gl", name="gin_gl",
                                 bufs=1)
                    nc.sync.dma_start(gl[:], u2T_d[k][:, :GS])
                    sc, sh = bn2[k]
                    nc.scalar.activation(gt[:], gl[:], ACTF.Identity,
                                         bias=sh[:], scale=sc[:])
                    ginT.append(gt)
            else:
                ginT = []
                for k in range(2):
                    gt = cp.tile([128, GS], F32, name=f"ginT{k}")
                    nc.vector.memset(gt[:], 0.0)
                    ginT.append(gt)

            # =========== Phase C: conformer edge pipeline ===================
            # zero agg_dram
            zt = cp.tile([128, 512], F32, name="zero_sb")
            nc.vector.memset(zt[:], 0.0)
            zrows = 0
            NSg = NS + 128
            while zrows < NSg:
                r = min(512, NSg - zrows)
                p = 128 if r >= 128 else r
                r = (r // p) * p
                ap = agg_dram[zrows:zrows + r, :].rearrange(
                    "(t p) f -> p t f", p=p)
                zs = zt[:p, :r * NF // p].rearrange("p (t f) -> p t f", f=NF)
                nc.sync.dma_start(ap, zs)
                zrows += r

            # resident: C row (cosine cutoff per edge, tile layout)
            crow = cp.tile([128, D.E_pad // 128], F32, name="crow_sb")
            for s0 in range(0, D.E_pad // 128, 512):
                sw = min(512, D.E_pad // 128 - s0)
                wt = wp.tile([128, 512], F32, tag="ph_c_wt", name="ph_c_wt",
                             bufs=1)
                nc.sync.dma_start(wt[:, :sw], I["WT"].ap()[:, s0:s0 + sw])
                nc.scalar.activation(wt[:, :sw], wt[:, :sw], ACTF.Sin,
                                     bias=C["pihalf"][:],
                                     scale=math.pi / D.CUTOFF)
                nc.scalar.activation(crow[:, s0:s0 + sw], wt[:, :sw],
                                     ACTF.Copy, bias=0.5, scale=-0.5)

            NT = D.chunk // 128
            chunks_per_q = D.EQ_pad // D.chunk
            ph_bounds = list(np.cumsum(D.phase_sizes))
            HC = D.chunk // 2  # half-chunk = one gather call
            NTH = HC // 128
            b2v = C["b2full"][:].rearrange("p (t f) -> p t f", f=NF)
            SIW = 8 * D.chunk  # si/dsti block (edges)
            for q in range(D.NQ if "c" in phases else 0):
                qlo = q * D.qsize
                qe0 = q * D.EQ_pad
                si = dsti = None
                for cc in range(chunks_per_q):
                    e0 = qe0 + cc * D.chunk
                    le0 = cc * D.chunk  # quad-local edge offset
                    if cc % 8 == 0:
                        sb0 = le0
                        sbn = min(SIW, D.EQ_pad - sb0)
                        si = wp.tile([128, SIW // 16], I16, tag="ph_c_si",
                                     name="ph_c_si")
                        nc.sync.dma_start(
                            si[:, :sbn // 16],
                            I["SRC"].ap()[:, (qe0 + sb0) // 16:
                                          (qe0 + sb0 + sbn) // 16])
                        dsti = wp.tile([128, SIW // 16], I16, tag="ph_c_di",
                                       name="ph_c_di")
                        nc.sync.dma_start(
                            dsti[:, :sbn // 16],
                            I["DSTI"].ap()[:, (qe0 + sb0) // 16:
                                           (qe0 + sb0 + sbn) // 16])
                    so = le0 - sb0  # offset within si/dsti block
                    msg = wp.tile([128, NT, NF], F32, tag="ph_c_msg",
                                  name="ph_c_msg")
                    for hh in range(2):
                        h0 = hh * HC
                        gat = wp.tile([128, NTH, NF], F32, tag="ph_c_gat",
                                      name="ph_c_gat")
                        nc.gpsimd.dma_gather(
                            gat[:], xf_full[qlo:qlo + D.qsize, :],
                            si[:, (so + h0) // 16:(so + h0 + HC) // 16],
                            num_idxs=HC, num_idxs_reg=HC, elem_size=NF)
                        at = wp.tile([NG, HC], MMDT, tag="ph_c_at",
                                     name="ph_c_at")
                        nc.sync.dma_start(
                            at[:], I["AT"].ap()[:, e0 + h0:e0 + h0 + HC])
                        h1 = wp.tile([128, HC], MMDT, tag="ph_c_h1",
                                     name="ph_c_h1")
                        for s0 in range(0, HC, 512):
                            sw = min(512, HC - s0)
                            ps1 = pp.tile([128, 512], F32, tag="ps_mm",
                                          name="ps_mm")
                            nc.tensor.matmul(ps1[:, :sw], C["w1"][:],
                                             at[:, s0:s0 + sw], start=True,
                                             stop=True)
                            nc.scalar.activation(h1[:, s0:s0 + sw], ps1[:, :sw],
                                                 ACTF.Relu, bias=C["b1col"][:])
                        for g0 in range(0, NTH, 4):
                            psw = pp.tile([128, 4, NF], F32, tag="ps_w",
                                          name="ps_w")
                            for ti in range(4):
                                t = g0 + ti
                                nc.tensor.matmul(psw[:, ti, :],
                                                 h1[:, _ts(t, 128)],
                                                 C["w2"][:], start=True,
                                                 stop=True)
                            wb = sp.tile([128, 4, NF], F32, tag="ph_c_wb",
                                         name="ph_c_wb")
                            nc.vector.tensor_tensor(wb[:], psw[:], b2v,
                                                    ALU.add)
                            cb = (e0 + h0) // 128 + g0
                            cr = crow[:, cb:cb + 4]
                            nc.vector.tensor_tensor(
                                wb[:], wb[:], cr.broadcast_to((128, 4, NF)),
                                ALU.mult)
                            nc.vector.tensor_tensor(
                                msg[:, hh * NTH + g0:hh * NTH + g0 + 4, :],
                                wb[:], gat[:, g0:g0 + 4, :], ALU.mult)
                    # scatter-add into agg: split calls at phase boundaries
                    c0 = cc * D.chunk
                    c1 = c0 + D.chunk
                    cuts = {c0, c1}
                    for b in ph_bounds:
                        if c0 < b < c1:
                            cuts.add(int(b))
                    cuts = sorted(cuts)
                    for a, b in zip(cuts[:-1], cuts[1:]):
                        for s in range(a, b, D.dcall):
                            sn = min(D.dcall, b - s)
                            la = s - c0
                            nc.gpsimd.dma_scatter_add(
                                agg_dram[:],
                                msg[:, la // 128:(la + sn) // 128, :],
                                dsti[:, (so + la) // 16:(so + la + sn) // 16],
                                num_idxs=sn, num_idxs_reg=sn, elem_size=NF)

            # =========== Phase D: h = relu(agg@lin2+b)@linw+b, residual =====
            NCH = D.nchunk
            n_nch = NS // NCH
            for j in range(n_nch):
                r0 = j * NCH
                aggT = wp.tile([NF, NCH], F32, tag="ph_d_aggT", name="ph_d_aggT")
                PB = 125  # NCH = 4 * PB, rows wrapped 125/partition
                asb = wp.tile([PB, 4, NF], F32, tag="ph_d_asb", name="ph_d_asb")
                nc.sync.dma_start(
                    asb[:],
                    agg_dram[r0:r0 + NCH, :].rearrange("(t p) f -> p t f",
                                                       p=PB))
                for t in range(4):
                    pst = pp.tile([128, 128], F32, tag="ps_tr", name="ps_tr")
                    nc.tensor.transpose(pst[:, :PB], asb[:PB, t, :],
                                        C["ident"][:PB, :PB])
                    nc.vector.tensor_copy(aggT[:, t * PB:(t + 1) * PB],
                                          pst[:, :PB])
                h1T = [wp.tile([128, NCH], F32, tag=f"ph_d_h1T{k}",
                               name=f"ph_d_h1T{k}")
                       for k in range(2)]
                for k in range(2):
                    ps = pp.tile([128, NCH], F32, tag="ps_mm", name="ps_mm")
                    nc.tensor.matmul(ps[:], C["lin2"][:, _ts(k, 128)], aggT[:],
                                     start=True, stop=True)
                    nc.scalar.activation(h1T[k][:], ps[:], ACTF.Relu,
                                         bias=C["lin2b"][k][:])
                for k in range(2):
                    ps = pp.tile([128, NCH], F32, tag="ps_mm", name="ps_mm")
                    for kk in range(2):
                        nc.tensor.matmul(ps[:], C["linw"][kk][:, _ts(k, 128)],
                                         h1T[kk][:], start=(kk == 0),
                                         stop=(kk == 1))
                    ob = sp.tile([128, NCH], F32, tag="ph_d_ob", name="ph_d_ob")
                    nc.scalar.activation(ob[:], ps[:], ACTF.Identity,
                                         bias=C["linb"][k][:])
                    xtc = sp.tile([128, NCH], F32, tag="ph_d_xtc", name="ph_d_xtc")
                    nc.scalar.dma_start(xtc[:], I["xT"].ap()[k, :, r0:r0 + NCH])
                    nc.vector.tensor_tensor(ob[:], ob[:], xtc[:], ALU.add)
                    g0 = r0 // rep
                    gin_rep = ginT[k][:, g0:g0 + NCH // rep].broadcast_to(
                        (128, NCH // rep, rep))
                    nc.vector.tensor_tensor(
                        ob[:].rearrange("p (g t) -> p g t", t=rep),
                        ob[:].rearrange("p (g t) -> p g t", t=rep),
                        gin_rep, ALU.add)
                    nc.sync.dma_start(outT.ap()[k, :, r0:r0 + NCH], ob[:])

    nc.compile()
    return nc


_CACHE = {}


def _get_nc(D: Dims, phases: str = "abcd"):
    key = ("nc", D, phases)
    if key not in _CACHE:
        _CACHE[key] = build_nc(D, phases)
    return _CACHE[key]


def run_on_hw(inputs, D: Dims = REAL):
    nc = _get_nc(D)
    in_maps = host_prep(inputs, D)
    res = bass_utils.run_bass_kernel_spmd(nc, in_maps,
                                          core_ids=list(range(D.cores)))
    return assemble(res.results, D)


def kernel(**inputs):
    return run_on_hw(inputs, REAL)


# revision 5
# speedup vs baseline: 1.2981x; 1.1228x over previous
"""Trainium2 Bass kernel for nn_DSSConf — v2: zero collectives.

Design: replicate the full x to every core, ROTATED by the core's node
offset so the SPMD program is fully static (each core's own shard is
always at local offset 0). Each core computes the full xf table and the
full (replicated) GIN branch locally, then processes its 1/8 of the
conformer edges and emits its output shard. No collectives at all.

Self-contained: hardcodes shapes/sharding; exposes kernel(**inputs).
"""
import sys
import math
from dataclasses import dataclass

sys.path.insert(0, "/opt/trn_rl_repo")

import numpy as np
from concourse import bass, bacc, tile, mybir, bass_utils

F32 = mybir.dt.float32
BF16 = mybir.dt.bfloat16
I16 = mybir.dt.int16
ALU = mybir.AluOpType
ACTF = mybir.ActivationFunctionType
AX = mybir.AxisListType

WMLP_BF16 = False  # edge-filter MLP matmuls in bf16 (flip after precision exp)
GIN_BF16 = False


@dataclass(frozen=True)
class Dims:
    N: int = 100000        # conformer nodes
    H: int = 256           # hidden
    NF: int = 128          # num filters
    NG: int = 50           # num gaussians
    G: int = 10000         # graph nodes
    E: int = 1000000       # conformer edges
    EG: int = 30000        # graph edges
    VOCAB: int = 5
    CUTOFF: float = 10.0
    cores: int = 8
    qsize: int = 25000     # src quadrant size for int16 gather indices
    chunk: int = 2048      # conformer edge chunk (multiple of 128, divides EQ_pad)
    dcall: int = 1024      # max descriptors per gather/scatter DMA call
                           # (SWDGE ring = dynamic_dma_scratch_size/16 = 1024)
    gwin: int = 128        # GIN scatter window (<=128 segments)
    PW: int = 512          # padded GIN edges per window (multiple of 128)
    GW: int = 1            # GIN windows per gather call
    nchunk: int = 500      # node chunk for the h/out stage (divides NS, mult of 10)
    pch: int = 1280        # Phase A node chunk (mult of 128 and of rep=10)

    @property
    def NS(self):
        return self.N // self.cores

    @property
    def GS(self):
        return self.G // self.cores

    @property
    def NQ(self):
        return (self.N + self.qsize - 1) // self.qsize

    @property
    def phase_sizes(self):
        """Fixed per-quadrant edge-bucket sizes, one per dst-occurrence
        rank. Within a bucket every dst is unique -> dma_scatter_add calls
        that stay inside a bucket are race-free. Sized from a Poisson model
        with margin; host_prep asserts the actual counts fit."""
        lam = (self.E / (self.cores * self.NQ)) / self.NS
        sizes = []
        pmf = math.exp(-lam)
        cdf = pmf
        p = 0
        while True:
            sf = 1.0 - cdf  # P(X >= p+1)
            mp = self.NS * sf
            if mp < 0.02 and p > 0:
                break
            pad = int(mp + 4.0 * math.sqrt(max(mp, 1.0)) + 64)
            sizes.append((pad + 127) // 128 * 128)
            p += 1
            pmf *= lam / p
            cdf += pmf
            if p > 64:
                break
        tot = sum(sizes)
        eq = (tot + self.chunk - 1) // self.chunk * self.chunk
        sizes[-1] += eq - tot
        return sizes

    @property
    def EQ_pad(self):
        return sum(self.phase_sizes)

    @property
    def E_pad(self):
        return self.NQ * self.EQ_pad

    @property
    def NWIN(self):
        return (self.G + self.gwin - 1) // self.gwin

    @property
    def EG_pad(self):
        return self.NWIN * self.PW


REAL = Dims()


def _wrap16(arr, dtype=np.int16):
    """Edge i -> [i % 16, i // 16], replicated to 128 partitions."""
    a = np.asarray(arr).reshape(-1, 16).T.astype(dtype)
    return np.tile(a, (8, 1)).copy()


def _tile128(arr, dtype=np.float32):
    """Edge i -> [i % 128, i // 128] (per-partition scalar layout)."""
    return np.ascontiguousarray(np.asarray(arr).reshape(-1, 128).T.astype(dtype))


def host_prep(inputs, D: Dims):
    """Build per-core in_maps (list of dicts) for the SPMD kernel."""
    x = np.asarray(inputs["x"], np.float32)
    cnb = np.asarray(inputs["conf_node_batch"]).astype(np.int64)
    ei = np.asarray(inputs["edge_index_conf"]).astype(np.int64)
    ew = np.asarray(inputs["edge_weight_conf"], np.float32)
    ea = np.asarray(inputs["edge_attr_conf"], np.float32)
    eig = np.asarray(inputs["edge_index_graph"]).astype(np.int64)
    eag = np.asarray(inputs["edge_attr_graph"]).astype(np.int64)

    rep = D.N // D.G
    assert np.array_equal(cnb, np.repeat(np.arange(D.G), rep)), \
        "conf_node_batch structure mismatch"

    NS, GS = D.NS, D.GS
    src, dst = ei[0], ei[1]
    owner = dst // NS
    sizes = D.phase_sizes
    ph_off = np.concatenate([[0], np.cumsum(sizes)])[:-1]

    xT = np.ascontiguousarray(x.T).reshape(2, 128, D.N)

    # GIN edges (global)
    sg, dg = eig[0], eig[1]

    # ---- weights (replicated) ----
    H2 = D.H // 128
    w = {k: np.asarray(inputs[k], np.float32) for k in (
        "mlp_w1", "mlp_b1", "mlp_w2", "mlp_b2", "cf_lin1", "cf_lin2",
        "cf_lin2_b", "lin_w", "lin_b", "bond_emb", "gin_eps", "gin_w1",
        "gin_w2", "bn1_g", "bn1_b", "bn2_g", "bn2_b")}
    mmdt = np.float32
    const = dict(
        w1=w["mlp_w1"].astype(mmdt),                      # [NG, NF]
        b1col=w["mlp_b1"].reshape(D.NF, 1),
        w2=w["mlp_w2"].astype(mmdt),                      # [NF, NF]
        b2full=np.tile(w["mlp_b2"].reshape(1, D.NF), (128, 4)).copy(),
        lin1=np.ascontiguousarray(w["cf_lin1"].reshape(H2, 128, D.NF)),
        lin2=w["cf_lin2"],                                # [NF, H]
        lin2b=w["cf_lin2_b"].reshape(H2, 128, 1),
        linw=np.ascontiguousarray(w["lin_w"].reshape(H2, 128, D.H)),
        linb=w["lin_b"].reshape(H2, 128, 1),
        gw1=np.ascontiguousarray(w["gin_w1"].reshape(H2, 128, D.H)),
        gw2=np.ascontiguousarray(w["gin_w2"].reshape(H2, 128, D.H)),
        bondcat=np.ascontiguousarray(
            w["bond_emb"].reshape(3 * D.VOCAB, D.H)),
        bn1g=w["bn1_g"].reshape(H2, 128, 1), bn1b=w["bn1_b"].reshape(H2, 128, 1),
        bn2g=w["bn2_g"].reshape(H2, 128, 1), bn2b=w["bn2_b"].reshape(H2, 128, 1),
        epsv=np.full((128, 1), 1.0 + float(w["gin_eps"]), np.float32),
        zerocol=np.zeros((128, 1), np.float32),
        eps5col=np.full((128, 1), 1e-5, np.float32),
        pihalf=np.full((128, 1), -math.pi / 2, np.float32),
        iota=np.tile(np.arange(128, dtype=np.float32), (128, 1)).copy(),
        ident=np.eye(128, dtype=np.float32),
    )

    in_maps = []
    for c in range(D.cores):
        # ---- rotated full x ----
        off = c * NS
        xr = np.concatenate([xT[:, :, off:], xT[:, :, :off]], axis=2)

        # ---- conformer edges owned by this core ----
        sel = owner == c
        s = src[sel]
        dd = dst[sel] - off                      # [0, NS)
        sr = (s - off) % D.N                     # rotated src
        q = sr // D.qsize
        order = np.lexsort((dd, q))
        s_sr, s_d, s_q = sr[order], dd[order], q[order]
        s_w = ew[sel][order]
        s_a = ea[sel][order]

        src_pad = np.zeros(D.E_pad, np.int64)
        dst_pad = NS + (np.arange(D.E_pad, dtype=np.int64) % 128)
        w_pad = np.full(D.E_pad, D.CUTOFF, np.float32)   # C(CUTOFF)=0 kills pads
        a_pad = np.zeros((D.E_pad, D.NG), np.float32)
        bounds = np.searchsorted(s_q, np.arange(D.NQ + 1))
        for qq in range(D.NQ):
            lo, hi = bounds[qq], bounds[qq + 1]
            d = s_d[lo:hi]                        # dst-sorted within the bucket
            rank = np.arange(len(d)) - np.searchsorted(d, d)
            counts = np.bincount(rank, minlength=len(sizes))
            assert len(counts) <= len(sizes) and (counts <= sizes).all(), \
                f"phase overflow: core {c} quad {qq}: {counts} vs {sizes}"
            o = qq * D.EQ_pad
            pos = np.empty(len(d), np.int64)
            for p in np.unique(rank):
                psel = rank == p
                pos[psel] = o + ph_off[p] + np.arange(counts[p])
            src_pad[pos] = s_sr[lo:hi] - qq * D.qsize
            dst_pad[pos] = d
            w_pad[pos] = s_w[lo:hi]
            a_pad[pos] = s_a[lo:hi]

        # ---- GIN edges (full graph, rotated) ----
        sgr = (sg - c * GS) % D.G
        dgr = (dg - c * GS) % D.G
        gw_ = dgr // D.gwin
        gorder = np.lexsort((dgr, gw_))
        g_s, g_d, g_w = sgr[gorder], dgr[gorder], gw_[gorder]
        g_a = eag[gorder]

        sg_pad = np.zeros(D.EG_pad, np.int64)
        dr_pad = np.full(D.EG_pad, -1.0, np.float32)  # -1 kills pads in one-hot
        bh_pad = np.zeros((3 * D.VOCAB, D.EG_pad), np.float32)
        gbounds = np.searchsorted(g_w, np.arange(D.NWIN + 1))
        for ww in range(D.NWIN):
            lo, hi = gbounds[ww], gbounds[ww + 1]
            cnt = hi - lo
            assert cnt <= D.PW, f"PW overflow: core {c} win {ww}: {cnt}"
            o = ww * D.PW
            sg_pad[o:o + cnt] = g_s[lo:hi]
            dr_pad[o:o + cnt] = (g_d[lo:hi] - ww * D.gwin).astype(np.float32)
            for k in range(3):
                bh_pad[k * D.VOCAB + g_a[lo:hi, k], np.arange(o, o + cnt)] = 1.0

        m = dict(
            xT=np.ascontiguousarray(xr),
            AT=np.ascontiguousarray(a_pad.T),
            WT=_tile128(w_pad),
            SRC=_wrap16(src_pad),
            DSTI=_wrap16(dst_pad),
            SG=_wrap16(sg_pad),
            DREL=_tile128(dr_pad),
            BHOT=bh_pad,
        )
        m.update(const)
        in_maps.append(m)
    return in_maps


def assemble(results, D: Dims):
    """Per-core outT [2,128,NS] -> full [N, H]."""
    parts = [r["outT"].reshape(D.H, D.NS) for r in results]
    outT = np.concatenate(parts, axis=1)  # [H, N]
    return np.ascontiguousarray(outT.T)


def _ts(i, n):
    return bass.ts(i, n)


def build_nc(D: Dims, phases: str = "abcd"):
    nc = bacc.Bacc("TRN2", target_bir_lowering=False, debug=False,
                   num_devices=D.cores)
    NS, GS, H, NF, NG, G = D.NS, D.GS, D.H, D.NF, D.NG, D.G
    H2 = H // 128
    rep = D.N // D.G
    MMDT = F32

    I = {}
    def di(name, shape, dt=F32):
        I[name] = nc.dram_tensor(name, list(shape), dt, kind="ExternalInput")
        return I[name]

    di("xT", [2, 128, D.N])
    di("AT", [NG, D.E_pad])
    di("WT", [128, D.E_pad // 128])
    di("SRC", [128, D.E_pad // 16], I16)
    di("DSTI", [128, D.E_pad // 16], I16)
    di("SG", [128, D.EG_pad // 16], I16)
    di("DREL", [128, D.EG_pad // 128])
    di("BHOT", [3 * D.VOCAB, D.EG_pad])
    di("w1", [NG, NF], MMDT); di("b1col", [NF, 1]); di("w2", [NF, NF], MMDT)
    di("b2full", [128, 4 * NF])
    di("lin1", [H2, 128, NF]); di("lin2", [NF, H]); di("lin2b", [H2, 128, 1])
    di("linw", [H2, 128, H]); di("linb", [H2, 128, 1])
    di("gw1", [H2, 128, H]); di("gw2", [H2, 128, H])
    di("bondcat", [3 * D.VOCAB, H])
    di("bn1g", [H2, 128, 1]); di("bn1b", [H2, 128, 1])
    di("bn2g", [H2, 128, 1]); di("bn2b", [H2, 128, 1])
    di("epsv", [128, 1]); di("iota", [128, 128]); di("ident", [128, 128])
    di("zerocol", [128, 1]); di("eps5col", [128, 1]); di("pihalf", [128, 1])

    outT = nc.dram_tensor("outT", [2, 128, NS], F32, kind="ExternalOutput")

    with tile.TileContext(nc) as tc:
        with (
            tc.tile_pool(name="const", bufs=1) as cp,
            tc.tile_pool(name="work", bufs=2) as wp,
            tc.tile_pool(name="small", bufs=2) as sp,
            tc.tile_pool(name="psum", bufs=2, space="PSUM") as pp,
            tc.tile_pool(name="dram", bufs=1, space="DRAM") as dp,
        ):
            # ---------- load constants ----------
            C = {}
            for nm, shp, dt in [("w1", [NG, NF], MMDT), ("b1col", [NF, 1], F32),
                                ("w2", [NF, NF], MMDT),
                                ("b2full", [128, 4 * NF], F32),
                                ("lin2", [NF, H], F32),
                                ("bondcat", [3 * D.VOCAB, H], F32),
                                ("epsv", [128, 1], F32), ("iota", [128, 128], F32),
                                ("ident", [128, 128], F32),
                                ("zerocol", [128, 1], F32),
                                ("eps5col", [128, 1], F32),
                                ("pihalf", [128, 1], F32)]:
                t = cp.tile(shp, dt, name=f"c_{nm}")
                nc.sync.dma_start(t[:], I[nm].ap())
                C[nm] = t
            nc.const_aps.aps[(F32, 0.0)] = C["zerocol"][:]
            for nm in ("lin1", "lin2b", "linw", "linb", "gw1", "gw2",
                       "bn1g", "bn1b", "bn2g", "bn2b"):
                C[nm] = []
                inner = I[nm].shape[2]
                for k in range(H2):
                    t = cp.tile([128, inner], F32, name=f"c_{nm}{k}")
                    nc.sync.dma_start(t[:], I[nm].ap()[k])
                    C[nm].append(t)

            # ---------- DRAM scratch (all core-local) ----------
            xf_full = dp.tile([D.N, NF], F32, name="xf_full")
            xaggT_d = dp.tile([2, 128, G], F32, name="xaggT_d")
            xagg_full = dp.tile([G, H], F32, name="xagg_full")
            tT_d = dp.tile([2, 128, G], F32, name="tT_d")
            u1T_d = dp.tile([2, 128, G], F32, name="u1T_d")
            u2T_d = dp.tile([2, 128, G], F32, name="u2T_d")
            agg_dram = dp.tile([NS + 128, NF], F32, name="agg_dram")

            # =========== Phase A: xf for ALL nodes + segment-max pool =======
            # Full chunks of PCH (mult of 128 and rep), ragged tail after.
            PCH = D.pch
            n_full = D.N // PCH if "a" in phases else 0
            NTA = PCH // 128
            for j in range(n_full):
                xt = [wp.tile([128, PCH], F32, tag=f"ph_a_xt{k}",
                              name=f"ph_a_xt{k}")
                      for k in range(2)]
                for k in range(2):
                    nc.sync.dma_start(xt[k][:], I["xT"].ap()[k, :, _ts(j, PCH)])
                # pool: max over groups of `rep` cols -> xaggT_d chunk cols
                for k in range(2):
                    xa_sb = sp.tile([128, PCH // rep], F32, tag="ph_a_poolsb",
                                    name="ph_a_poolsb")
                    nc.vector.tensor_reduce(
                        xa_sb[:],
                        xt[k][:].rearrange("p (g t) -> p g t", t=rep),
                        AX.X, ALU.max)
                    nc.sync.dma_start(
                        xaggT_d[k][:, _ts(j, PCH // rep)], xa_sb[:])
                # xf = x @ cf_lin1: mm pairs into 4-tile psum banks, one
                # bulk row-wrapped write per chunk
                sb = wp.tile([128, NTA, NF], F32, tag="ph_a_sb", name="ph_a_sb")
                for g0 in range(0, NTA, 4):
                    gn = min(4, NTA - g0)
                    ps = pp.tile([128, 4, NF], F32, tag="ps_mm", name="ps_mm")
                    for ti in range(gn):
                        t = g0 + ti
                        for k in range(2):
                            nc.tensor.matmul(ps[:, ti, :],
                                             xt[k][:, t * 128:(t + 1) * 128],
                                             C["lin1"][k][:], start=(k == 0),
                                             stop=(k == 1))
                    nc.scalar.copy(sb[:, g0:g0 + gn, :], ps[:, :gn, :])
                nc.scalar.dma_start(
                    xf_full[j * PCH:(j + 1) * PCH, :].rearrange(
                        "(t p) f -> p t f", p=128),
                    sb[:])
            # ragged tail (node-at-a-time tiles)
            tail0 = n_full * PCH
            n_tail = D.N - tail0 if "a" in phases else 0
            if n_tail:
                xt = [wp.tile([128, n_tail], F32, tag=f"ph_a_xt{k}",
                              name=f"ph_a_xtt{k}")
                      for k in range(2)]
                for k in range(2):
                    nc.sync.dma_start(xt[k][:],
                                      I["xT"].ap()[k, :, tail0:tail0 + n_tail])
                for k in range(2):
                    xa_sb = sp.tile([128, n_tail // rep], F32,
                                    tag="ph_a_poolsb", name="ph_a_poolsbt")
                    nc.vector.tensor_reduce(
                        xa_sb[:],
                        xt[k][:].rearrange("p (g t) -> p g t", t=rep),
                        AX.X, ALU.max)
                    nc.sync.dma_start(
                        xaggT_d[k][:, tail0 // rep:(tail0 + n_tail) // rep],
                        xa_sb[:])
                for t in range((n_tail + 127) // 128):
                    m = min(128, n_tail - t * 128)
                    ps = pp.tile([128, 4, NF], F32, tag="ps_mm", name="ps_mm")
                    for k in range(2):
                        nc.tensor.matmul(ps[:m, 0, :],
                                         xt[k][:, t * 128:t * 128 + m],
                                         C["lin1"][k][:], start=(k == 0),
                                         stop=(k == 1))
                    sb = sp.tile([128, NF], F32, tag="ph_a_tsb", name="ph_a_tsb")
                    nc.scalar.copy(sb[:m, :], ps[:m, 0, :])
                    nc.sync.dma_start(
                        xf_full[tail0 + t * 128: tail0 + t * 128 + m, :],
                        sb[:m, :])

            # =========== Phase A2: transpose xagg -> node-major =============
            GT = (G + 127) // 128
            for t in range(GT if "a" in phases else 0):
                m = min(128, G - t * 128)
                for k in range(2):
                    la = sp.tile([128, 128], F32, tag="ph_a2_la", name="ph_a2_la")
                    nc.scalar.dma_start(la[:, :m],
                                        xaggT_d[k][:, t * 128:t * 128 + m])
                    pst = pp.tile([128, 128], F32, tag="ps_tr", name="ps_tr")
                    nc.tensor.transpose(pst[:m, :], la[:, :m], C["ident"][:])
                    sb = sp.tile([128, 128], F32, tag="ph_a2_sb", name="ph_a2_sb")
                    nc.scalar.copy(sb[:m, :], pst[:m, :])
                    nc.sync.dma_start(
                        xagg_full[t * 128:t * 128 + m, _ts(k, 128)], sb[:m, :])

            # =========== Phase B: GIN message aggregation (full graph) ======
            sgidx = cp.tile([128, D.EG_pad // 16], I16, name="sgidx_sb")
            nc.sync.dma_start(sgidx[:], I["SG"].ap())
            drel = cp.tile([128, D.EG_pad // 128], F32, name="drel_sb")
            nc.sync.dma_start(drel[:], I["DREL"].ap())

            tiles_per_win = D.PW // 128
            iota_bc = C["iota"][:].rearrange(
                "p (o f) -> p o f", o=1).broadcast_to(
                    (128, tiles_per_win, D.gwin))

            def emit_b_window(w):
                m = min(D.gwin, G - w * D.gwin)
                gath_g = wp.tile([128, tiles_per_win, H], F32,
                                 tag="ph_b_gath", name="ph_b_gath")
                nc.gpsimd.dma_gather(
                    gath_g[:], xagg_full[:],
                    sgidx[:, w * D.PW // 16:(w + 1) * D.PW // 16],
                    num_idxs=D.PW, num_idxs_reg=D.PW, elem_size=H)
                bhot = wp.tile([3 * D.VOCAB, D.PW], F32, tag="ph_b_bhot",
                               name="ph_b_bhot")
                nc.scalar.dma_start(bhot[:],
                                    I["BHOT"].ap()[:, _ts(w, D.PW)])
                # one-hot for the whole window in one op
                oh = sp.tile([128, tiles_per_win, D.gwin], F32, tag="ph_b_oh",
                             name="ph_b_oh")
                dr = drel[:, w * tiles_per_win:(w + 1) * tiles_per_win]
                nc.vector.tensor_tensor(
                    oh[:], iota_bc,
                    dr.rearrange("p (t o) -> p t o", o=1).broadcast_to(
                        (128, tiles_per_win, D.gwin)),
                    ALU.is_equal)
                # msg = relu(gathered + bond_emb), 2-tile psum groups
                msg = wp.tile([128, tiles_per_win, H], F32, tag="ph_b_msg",
                              name="ph_b_msg")
                for i0 in range(0, tiles_per_win, 2):
                    gn = min(2, tiles_per_win - i0)
                    ps_emb = pp.tile([128, 2, H], F32, tag="ps_w",
                                     name="ps_emb")
                    for j in range(gn):
                        nc.tensor.matmul(ps_emb[:, j, :],
                                         bhot[:, _ts(i0 + j, 128)],
                                         C["bondcat"][:], start=True,
                                         stop=True)
                    wbt = sp.tile([128, 2, H], F32, tag="ph_b_wbt",
                                  name="ph_b_wbt")
                    nc.vector.tensor_tensor(wbt[:, :gn, :],
                                            gath_g[:, i0:i0 + gn, :],
                                            ps_emb[:, :gn, :], ALU.add)
                    nc.scalar.activation(msg[:, i0:i0 + gn, :],
                                         wbt[:, :gn, :], ACTF.Relu)
                # feature-major agg via onehot matmuls: psT[k] [128, m]
                # (ps_tr tag: idle during the B/C interleave)
                psT = [pp.tile([128, 128], F32, tag="ps_tr", name="ps_aggT0"),
                       pp.tile([128, 128], F32, tag="ps_tr", name="ps_aggT1")]
                for k in range(2):
                    for i in range(tiles_per_win):
                        nc.tensor.matmul(psT[k][:, :m],
                                         msg[:, i, _ts(k, 128)],
                                         oh[:, i, :m], start=(i == 0),
                                         stop=(i == tiles_per_win - 1))
                # tn[k] = (1+eps) * xaggT + agg_gT, straight to tT_d
                for k in range(2):
                    xa = sp.tile([128, 128], F32, tag="ph_b_xa", name="ph_b_xa")
                    nc.sync.dma_start(
                        xa[:, :m], xaggT_d[k][:, w * D.gwin:w * D.gwin + m])
                    tn = sp.tile([128, 128], F32, tag="ph_b_tn", name="ph_b_tn")
                    nc.vector.tensor_scalar(tn[:, :m], xa[:, :m],
                                            C["epsv"][:, :], None, ALU.mult)
                    nc.vector.tensor_tensor(tn[:, :m], tn[:, :m],
                                            psT[k][:, :m], ALU.add)
                    nc.sync.dma_start(
                        tT_d[k][:, w * D.gwin:w * D.gwin + m], tn[:, :m])

            # =========== GIN GEMM passes (streamed through DRAM) ============
            def bn_coeffs(st_acc, g_c, b_c, label):
                """scale/shift [128,1] per half from accumulated stats."""
                inv_n = 1.0 / float(G)
                out = []
                for k in range(2):
                    mu = sp.tile([128, 1], F32, tag=f"{label}_mu{k}",
                                 name=f"{label}_mu{k}")
                    nc.vector.tensor_scalar(mu[:], st_acc[:, 2 * k:2 * k + 1],
                                            inv_n, None, ALU.mult)
                    var = sp.tile([128, 1], F32, tag=f"{label}_va{k}",
                                  name=f"{label}_va{k}")
                    nc.vector.tensor_scalar(var[:],
                                            st_acc[:, 2 * k + 1:2 * k + 2],
                                            inv_n, None, ALU.mult)
                    mu2 = sp.tile([128, 1], F32, tag=f"{label}_m2{k}",
                                  name=f"{label}_m2{k}")
                    nc.vector.tensor_tensor(mu2[:], mu[:], mu[:], ALU.mult)
                    nc.vector.tensor_tensor(var[:], var[:], mu2[:],
                                            ALU.subtract)
                    sd = sp.tile([128, 1], F32, tag=f"{label}_sd{k}",
                                 name=f"{label}_sd{k}")
                    nc.scalar.activation(sd[:], var[:], ACTF.Sqrt,
                                         bias=C["eps5col"][:])
                    rs = sp.tile([128, 1], F32, tag=f"{label}_rs{k}",
                                 name=f"{label}_rs{k}")
                    nc.vector.reciprocal(rs[:], sd[:])
                    sc = cp.tile([128, 1], F32, name=f"{label}_sc{k}")
                    nc.vector.tensor_tensor(sc[:], g_c[k][:], rs[:], ALU.mult)
                    sh = cp.tile([128, 1], F32, name=f"{label}_sh{k}")
                    nc.vector.tensor_tensor(sh[:], mu[:], sc[:], ALU.mult)
                    nc.vector.tensor_tensor(sh[:], b_c[k][:], sh[:],
                                            ALU.subtract)
                    out.append((sc, sh))
                return out

            def gin_gemm(inT_d, Wc, outT_d, label, pre=None):
                """outT = W^T @ (pre(inT)) block-streamed, feature-major
                throughout (no transposes). Wc[k] is [128, H] = W rows
                [k*128:(k+1)*128, :]. Returns the stats accumulator."""
                st_acc = cp.tile([128, 4], F32, name=f"{label}_stacc")
                nc.vector.memset(st_acc[:], 0.0)
                BW = 512
                for b0 in range(0, G, BW):
                    bw = min(BW, G - b0)
                    tt = []
                    for k in range(2):
                        lt = wp.tile([128, BW], F32, tag=f"gmm_lt{k}",
                                     name=f"{label}_lt{k}")
                        nc.sync.dma_start(lt[:, :bw],
                                          inT_d[k][:, b0:b0 + bw])
                        if pre is not None:
                            sc, sh = pre[k]
                            nc.scalar.activation(lt[:, :bw], lt[:, :bw],
                                                 ACTF.Relu, bias=sh[:],
                                                 scale=sc[:])
                        tt.append(lt)
                    for fj in range(2):
                        ps = pp.tile([128, BW], F32, tag="ps_agg",
                                     name="ps_gmm")
                        for k in range(2):
                            nc.tensor.matmul(ps[:, :bw],
                                             Wc[k][:, _ts(fj, 128)],
                                             tt[k][:, :bw],
                                             start=(k == 0), stop=(k == 1))
                        ut = sp.tile([128, BW], F32, tag="gmm_ut",
                                     name=f"{label}_ut")
                        nc.scalar.copy(ut[:, :bw], ps[:, :bw])
                        nc.scalar.dma_start(
                            outT_d[fj][:, b0:b0 + bw], ut[:, :bw])
                        # stats accumulate
                        r1 = sp.tile([128, 1], F32, tag="gmm_r1",
                                     name=f"{label}_r1")
                        nc.vector.tensor_reduce(r1[:], ut[:, :bw], AX.X,
                                                ALU.add)
                        nc.vector.tensor_tensor(st_acc[:, 2 * fj:2 * fj + 1],
                                                st_acc[:, 2 * fj:2 * fj + 1],
                                                r1[:], ALU.add)
                        sq = sp.tile([128, BW], F32, tag="gmm_sq",
                                     name=f"{label}_sq")
                        nc.vector.tensor_tensor(sq[:, :bw], ut[:, :bw],
                                                ut[:, :bw], ALU.mult)
                        nc.vector.tensor_reduce(r1[:], sq[:, :bw], AX.X,
                                                ALU.add)
                        nc.vector.tensor_tensor(
                            st_acc[:, 2 * fj + 1:2 * fj + 2],
                            st_acc[:, 2 * fj + 1:2 * fj + 2],
                            r1[:], ALU.add)
                return st_acc

            def emit_gin_gemms():
                """GEMM passes + bn2 slice; call AFTER all B windows
                (reads tT_d)."""
                ginT = []
                if "b" in phases:
                    st1 = gin_gemm(tT_d, C["gw1"], u1T_d, "gmm1")
                    bn1 = bn_coeffs(st1, C["bn1g"], C["bn1b"], "bn1")
                    st2 = gin_gemm(u1T_d, C["gw2"], u2T_d, "gmm2", pre=bn1)
                    bn2 = bn_coeffs(st2, C["bn2g"], C["bn2b"], "bn2")
                    # core's gin shard (first GS cols after rotation)
                    for k in range(2):
                        gt = cp.tile([128, GS], F32, name=f"ginT{k}")
                        gl = sp.tile([128, GS], F32, tag="gin_gl",
                                     name="gin_gl", bufs=1)
                        nc.sync.dma_start(gl[:], u2T_d[k][:, :GS])
                        sc, sh = bn2[k]
                        nc.scalar.activation(gt[:], gl[:], ACTF.Identity,
                                             bias=sh[:], scale=sc[:])
                        ginT.append(gt)
                else:
                    for k in range(2):
                        gt = cp.tile([128, GS], F32, name=f"ginT{k}")
                        nc.vector.memset(gt[:], 0.0)
                        ginT.append(gt)
                return ginT

            # =========== Phase C setup: zero agg, cosine-cutoff row =========
            zt = cp.tile([128, 512], F32, name="zero_sb")
            nc.vector.memset(zt[:], 0.0)
            zrows = 0
            NSg = NS + 128
            while zrows < NSg:
                r = min(512, NSg - zrows)
                p = 128 if r >= 128 else r
                r = (r // p) * p
                ap = agg_dram[zrows:zrows + r, :].rearrange(
                    "(t p) f -> p t f", p=p)
                zs = zt[:p, :r * NF // p].rearrange("p (t f) -> p t f", f=NF)
                nc.sync.dma_start(ap, zs)
                zrows += r

            crow = cp.tile([128, D.E_pad // 128], F32, name="crow_sb")
            wt = wp.tile([128, D.E_pad // 128], F32, tag="ph_c_wt",
                         name="ph_c_wt", bufs=1)
            nc.sync.dma_start(wt[:], I["WT"].ap())
            nc.scalar.activation(wt[:], wt[:], ACTF.Sin,
                                 bias=C["pihalf"][:], scale=math.pi / D.CUTOFF)
            nc.scalar.activation(crow[:], wt[:], ACTF.Copy, bias=0.5,
                                 scale=-0.5)

            NT = D.chunk // 128
            chunks_per_q = D.EQ_pad // D.chunk
            ph_bounds = list(np.cumsum(D.phase_sizes))
            HC = D.chunk // 2  # half-chunk = one gather call
            NTH = HC // 128
            b2v = C["b2full"][:].rearrange("p (t f) -> p t f", f=NF)
            SIW = 8 * D.chunk  # si/dsti block (edges)
            cst = {}  # per-quad si/dsti block state

            def emit_c_chunk(gc):
                q, cc = gc // chunks_per_q, gc % chunks_per_q
                qlo = q * D.qsize
                qe0 = q * D.EQ_pad
                e0 = qe0 + cc * D.chunk
                le0 = cc * D.chunk  # quad-local edge offset
                if cc % 8 == 0:
                    cst["sb0"] = le0
                    sbn = min(SIW, D.EQ_pad - le0)
                    si = wp.tile([128, SIW // 16], I16, tag="ph_c_si",
                                 name="ph_c_si")
                    nc.sync.dma_start(
                        si[:, :sbn // 16],
                        I["SRC"].ap()[:, (qe0 + le0) // 16:
                                      (qe0 + le0 + sbn) // 16])
                    dsti = wp.tile([128, SIW // 16], I16, tag="ph_c_di",
                                   name="ph_c_di")
                    nc.sync.dma_start(
                        dsti[:, :sbn // 16],
                        I["DSTI"].ap()[:, (qe0 + le0) // 16:
                                       (qe0 + le0 + sbn) // 16])
                    cst["si"], cst["dsti"] = si, dsti
                si, dsti = cst["si"], cst["dsti"]
                so = le0 - cst["sb0"]  # offset within si/dsti block
                msg = wp.tile([128, NT, NF], F32, tag="ph_c_msg",
                              name="ph_c_msg")
                for hh in range(2):
                    h0 = hh * HC
                    gat = wp.tile([128, NTH, NF], F32, tag="ph_c_gat",
                                  name="ph_c_gat", bufs=3)
                    nc.gpsimd.dma_gather(
                        gat[:], xf_full[qlo:qlo + D.qsize, :],
                        si[:, (so + h0) // 16:(so + h0 + HC) // 16],
                        num_idxs=HC, num_idxs_reg=HC, elem_size=NF)
                    at = wp.tile([NG, HC], MMDT, tag="ph_c_at",
                                 name="ph_c_at")
                    nc.sync.dma_start(
                        at[:], I["AT"].ap()[:, e0 + h0:e0 + h0 + HC])
                    h1 = wp.tile([128, HC], MMDT, tag="ph_c_h1",
                                 name="ph_c_h1")
                    for s0 in range(0, HC, 512):
                        sw = min(512, HC - s0)
                        ps1 = pp.tile([128, 512], F32, tag="ps_mm",
                                      name="ps_mm")
                        nc.tensor.matmul(ps1[:, :sw], C["w1"][:],
                                         at[:, s0:s0 + sw], start=True,
                                         stop=True)
                        nc.scalar.activation(h1[:, s0:s0 + sw], ps1[:, :sw],
                                             ACTF.Relu, bias=C["b1col"][:])
                    for g0 in range(0, NTH, 4):
                        psw = pp.tile([128, 4, NF], F32, tag="ps_agg",
                                      name="ps_w")
                        for ti in range(4):
                            t = g0 + ti
                            nc.tensor.matmul(psw[:, ti, :],
                                             h1[:, _ts(t, 128)],
                                             C["w2"][:], start=True,
                                             stop=True)
                        wb = sp.tile([128, 4, NF], F32, tag="ph_c_wb",
                                     name="ph_c_wb")
                        nc.vector.tensor_tensor(wb[:], psw[:], b2v,
                                                ALU.add)
                        cb = (e0 + h0) // 128 + g0
                        cr = crow[:, cb:cb + 4]
                        nc.vector.tensor_tensor(
                            wb[:], wb[:], cr.broadcast_to((128, 4, NF)),
                            ALU.mult)
                        nc.vector.tensor_tensor(
                            msg[:, hh * NTH + g0:hh * NTH + g0 + 4, :],
                            wb[:], gat[:, g0:g0 + 4, :], ALU.mult)
                # scatter-add into agg: split calls at phase boundaries
                c0 = cc * D.chunk
                c1 = c0 + D.chunk
                cuts = {c0, c1}
                for b in ph_bounds:
                    if c0 < b < c1:
                        cuts.add(int(b))
                cuts = sorted(cuts)
                for a, b in zip(cuts[:-1], cuts[1:]):
                    for s in range(a, b, D.dcall):
                        sn = min(D.dcall, b - s)
                        la = s - c0
                        nc.gpsimd.dma_scatter_add(
                            agg_dram[:],
                            msg[:, la // 128:(la + sn) // 128, :],
                            dsti[:, (so + la) // 16:(so + la + sn) // 16],
                            num_idxs=sn, num_idxs_reg=sn, elem_size=NF)

            # ---- interleave B windows with C chunks: B is TE/DVE-heavy,
            # ---- C is Pool/DMA-heavy; alternating emission lets each
            # ---- engine fill the other phase's dependency stalls.
            n_b = D.NWIN if "b" in phases else 0
            n_c = D.NQ * chunks_per_q if "c" in phases else 0
            for i in range(max(n_b, n_c)):
                if i < n_b:
                    emit_b_window(i)
                if i < n_c:
                    emit_c_chunk(i)

            # GIN GEMM passes overlap C's tail (Pool/DMA-heavy scatters)
            ginT = emit_gin_gemms()

            # =========== Phase D: h = relu(agg@lin2+b)@linw+b, residual =====
            NCH = D.nchunk
            n_nch = NS // NCH
            for j in range(n_nch):
                r0 = j * NCH
                aggT = wp.tile([NF, NCH], F32, tag="ph_d_aggT", name="ph_d_aggT")
                PB = 125  # NCH = 4 * PB, rows wrapped 125/partition
                asb = wp.tile([PB, 4, NF], F32, tag="ph_d_asb", name="ph_d_asb")
                nc.sync.dma_start(
                    asb[:],
                    agg_dram[r0:r0 + NCH, :].rearrange("(t p) f -> p t f",
                                                       p=PB))
                for t in range(4):
                    pst = pp.tile([128, 128], F32, tag="ps_tr", name="ps_tr")
                    nc.tensor.transpose(pst[:, :PB], asb[:PB, t, :],
                                        C["ident"][:PB, :PB])
                    nc.vector.tensor_copy(aggT[:, t * PB:(t + 1) * PB],
                                          pst[:, :PB])
                h1T = [wp.tile([128, NCH], F32, tag=f"ph_d_h1T{k}",
                               name=f"ph_d_h1T{k}")
                       for k in range(2)]
                for k in range(2):
                    ps = pp.tile([128, NCH], F32, tag="ps_mm", name="ps_mm")
                    nc.tensor.matmul(ps[:], C["lin2"][:, _ts(k, 128)], aggT[:],
                                     start=True, stop=True)
                    nc.scalar.activation(h1T[k][:], ps[:], ACTF.Relu,
                                         bias=C["lin2b"][k][:])
                for k in range(2):
                    ps = pp.tile([128, NCH], F32, tag="ps_mm", name="ps_mm")
                    for kk in range(2):
                        nc.tensor.matmul(ps[:], C["linw"][kk][:, _ts(k, 128)],
                                         h1T[kk][:], start=(kk == 0),
                                         stop=(kk == 1))
                    ob = sp.tile([128, NCH], F32, tag="ph_d_ob", name="ph_d_ob")
                    nc.scalar.activation(ob[:], ps[:], ACTF.Identity,
                                         bias=C["linb"][k][:])
                    xtc = sp.tile([128, NCH], F32, tag="ph_d_xtc", name="ph_d_xtc")
                    nc.scalar.dma_start(xtc[:], I["xT"].ap()[k, :, r0:r0 + NCH])
                    nc.vector.tensor_tensor(ob[:], ob[:], xtc[:], ALU.add)
                    g0 = r0 // rep
                    gin_rep = ginT[k][:, g0:g0 + NCH // rep].broadcast_to(
                        (128, NCH // rep, rep))
                    nc.vector.tensor_tensor(
                        ob[:].rearrange("p (g t) -> p g t", t=rep),
                        ob[:].rearrange("p (g t) -> p g t", t=rep),
                        gin_rep, ALU.add)
                    nc.sync.dma_start(outT.ap()[k, :, r0:r0 + NCH], ob[:])

    nc.compile()
    return nc


_CACHE = {}


def _get_nc(D: Dims, phases: str = "abcd"):
    key = ("nc", D, phases)
    if key not in _CACHE:
        _CACHE[key] = build_nc(D, phases)
    return _CACHE[key]


def run_on_hw(inputs, D: Dims = REAL):
    nc = _get_nc(D)
    in_maps = host_prep(inputs, D)
    res = bass_utils.run_bass_kernel_spmd(nc, in_maps,
                                          core_ids=list(range(D.cores)))
    return assemble(res.results, D)


def kernel(**inputs):
    return run_on_hw(inputs, REAL)
